# revision 1
# baseline (speedup 1.0000x reference)
"""DeepSeek sparse attention on 8 Trainium2 NeuronCores (Bass/Tile).

Strategy (3 SPMD launches, column/head-parallel, float32r matmuls):

  L1  (column-parallel): each core computes a 256-column slice of the three
      projections, emitted transposed: qT/kT/vT slices (256, S) from
      hidden^T (resident in SBUF) and the core's weight column slice.
  host: concat slices -> q_lin^T, k_lin^T, v_lin^T (H, S).
  L2  (indexer-head-parallel): core c owns indexer head c. Computes
      qp_c^T, kp_c^T (256, S) from full q_lin^T / k_lin^T, then
      rel_c[q] = sum_k relu(qp_c[q] . kp_c[k]) via PE + fused relu-accum.
  host: rel = sum_c w_c * rel_c * exp(-T); top-1024 keys -> selected mask;
      hi[k] = selected ? BIG : k + LOCAL_WINDOW (fp16 threshold vector).
  L3  (attention-head-parallel): core c owns attention heads 2c, 2c+1.
      scores^T per head via PE (f32r), exp via ACT (fp16), causal/local/
      selected masking via two fused iota-compare-multiply DVE ops,
      denominator via ones-matmul, normalize, out rows = ao @ Wo[head rows]
      -> per-core partial (S, H).
  host: out = sum_c partial_c.

Matmuls run as float32r (full PE rate at N>=512, ~1.5e-4 rel err).
"""

import math

import numpy as np

import concourse.bass as bass
import concourse.mybir as mybir
from concourse import bacc
from concourse.tile import TileContext
from concourse.masks import make_identity
from concourse.bass_utils import run_bass_kernel_spmd

# Problem constants (hardcoded per contract)
HIDDEN = 2048
NUM_HEADS = 16
HEAD_DIM = 128
NUM_IND_HEADS = 8
IND_DIM = HIDDEN // NUM_IND_HEADS  # 256
MAX_SELECTED = 1024
LOCAL_WINDOW = 512
N_CORES = 8

F32 = mybir.dt.float32
F32R = mybir.dt.float32r
F16 = mybir.dt.float16
BF16 = mybir.dt.bfloat16
FP32 = np.float32

_TRACE = {"on": False, "exec_ns": []}


def _bc(ap):
    return ap.bitcast(F32R)


def build_l1(S=2048, H=HIDDEN, CS=HIDDEN // N_CORES):
    """Per-core: qT/kT/vT (CS, S) = (W[:, cols].T @ hidden.T) for 3 weights."""
    nc = bacc.Bacc("TRN2", target_bir_lowering=False, debug=False)
    HT, MC, NQ = H // 128, CS // 128, S // 512
    hidT = nc.dram_tensor("hidT", [H, S], F32R, kind="ExternalInput")
    wq = nc.dram_tensor("wq", [H, CS], F32R, kind="ExternalInput")
    wk = nc.dram_tensor("wk", [H, CS], F32R, kind="ExternalInput")
    wv = nc.dram_tensor("wv", [H, CS], F32R, kind="ExternalInput")
    qT = nc.dram_tensor("qT", [CS, S], F32, kind="ExternalOutput")
    kT = nc.dram_tensor("kT", [CS, S], F32, kind="ExternalOutput")
    vT = nc.dram_tensor("vT", [CS, S], F32, kind="ExternalOutput")

    with TileContext(nc) as tc:
        with (
            tc.tile_pool(name="hid", bufs=1) as hpool,
            tc.tile_pool(name="wt", bufs=4) as wpool,
            tc.tile_pool(name="ev", bufs=4) as opool,
            tc.tile_pool(name="ps", bufs=2, space="PSUM") as pspool,
        ):
            # hidden^T resident, loaded as 8 chunks of 2 k-strips so the first
            # matmuls only wait on chunk 0 (~2 MB), not the full 16 MB.
            G = 8
            TG = HT // G

            def load_hidc(g):
                hc = hpool.tile([128, TG * S], F32R, name=f"hidc{g}")
                nc.sync.dma_start(
                    out=hc.rearrange("p (t s) -> p t s", t=TG),
                    in_=hidT[g * TG * 128:(g + 1) * TG * 128, :].rearrange(
                        "(t p) s -> p t s", p=128
                    ),
                )
                return hc

            def load_wres(wdram):
                # weight column-slice resident: one 2 MB DMA per projection.
                wr = wpool.tile([128, HT * CS], F32R, tag="wres", name="wres")
                nc.sync.dma_start(
                    out=wr.rearrange("p (t c) -> p t c", t=HT),
                    in_=wdram.rearrange("(t p) c -> p t c", p=128),
                )
                return wr

            hidc = [load_hidc(0)]
            wres = {wq.name: load_wres(wq)}
            hidc += [load_hidc(g) for g in range(1, G)]
            wres[wk.name] = load_wres(wk)
            wres[wv.name] = load_wres(wv)

            for wdram, odram in ((wq, qT), (wk, kT), (wv, vT)):
                wr = wres[wdram.name]
                for mc in range(MC):
                    psums = [
                        pspool.tile([128, 512], F32, tag=f"ps{qc}", name=f"ps{qc}")
                        for qc in range(NQ)
                    ]
                    for t in range(HT):
                        lhsT = wr[:, t * CS + mc * 128: t * CS + mc * 128 + 128]
                        rhs_tile = hidc[t // TG]
                        tl = t % TG
                        for qc in range(NQ):
                            nc.tensor.matmul(
                                psums[qc], lhsT,
                                rhs_tile[:, tl * S + qc * 512: tl * S + qc * 512 + 512],
                                start=(t == 0), stop=(t == HT - 1),
                            )
                    for qc in range(NQ):
                        ot = opool.tile([128, 512], F32, tag="ot", name="ot")
                        nc.scalar.copy(ot, psums[qc])
                        nc.sync.dma_start(
                            out=odram[mc * 128:(mc + 1) * 128, qc * 512:(qc + 1) * 512],
                            in_=ot,
                        )
    nc.compile()
    return nc


def build_l2(S=2048, H=HIDDEN, D=IND_DIM):
    """Per-core (indexer head c): rel_c[q] = sum_k relu(qp_c[q] . kp_c[k])."""
    nc = bacc.Bacc("TRN2", target_bir_lowering=False, debug=False)
    HT, DC, NQ, QT = H // 128, D // 128, S // 512, S // 128
    qTd = nc.dram_tensor("qT", [H, S], F32R, kind="ExternalInput")
    kTd = nc.dram_tensor("kT", [H, S], F32R, kind="ExternalInput")
    wqi = nc.dram_tensor("wqi", [H, D], F32R, kind="ExternalInput")
    wki = nc.dram_tensor("wki", [H, D], F32R, kind="ExternalInput")
    rel = nc.dram_tensor("rel", [S], F32, kind="ExternalOutput")

    with TileContext(nc) as tc:
        with (
            tc.tile_pool(name="strip", bufs=3) as spool,
            tc.tile_pool(name="wstrip", bufs=3) as wpool,
            tc.tile_pool(name="proj", bufs=1) as ppool,
            tc.tile_pool(name="scr", bufs=3) as scpool,
            tc.tile_pool(name="rc", bufs=2) as rcpool,
            tc.tile_pool(name="rm", bufs=1) as rmpool,
            tc.tile_pool(name="ps", bufs=1, space="PSUM") as pspool,
        ):
            qpt = [ppool.tile([128, S], F32R, name=f"qpt{mc}") for mc in range(DC)]
            kpt = [ppool.tile([128, S], F32R, name=f"kpt{mc}") for mc in range(DC)]
            wires = {}
            for wd in (wqi, wki):
                wr = wpool.tile([128, HT * D], F32R, tag="wires", name="wires")
                nc.sync.dma_start(
                    out=wr.rearrange("p (t c) -> p t c", t=HT),
                    in_=wd.rearrange("(t p) c -> p t c", p=128),
                )
                wires[wd.name] = wr
            for xTd, wd, dst in ((qTd, wqi, qpt), (kTd, wki, kpt)):
                wr = wires[wd.name]
                psq = [
                    pspool.tile([128, 512], F32, tag=f"m{i}", name=f"m{i}")
                    for i in range(DC * NQ)
                ]
                for t in range(HT):
                    xs = spool.tile([128, S], F32R, tag="xs", name="xs")
                    nc.sync.dma_start(out=xs, in_=xTd[t * 128:(t + 1) * 128, :])
                    for mc in range(DC):
                        for qc in range(NQ):
                            nc.tensor.matmul(
                                psq[mc * NQ + qc],
                                wr[:, t * D + mc * 128: t * D + mc * 128 + 128],
                                xs[:, qc * 512:(qc + 1) * 512],
                                start=(t == 0), stop=(t == HT - 1),
                            )
                for mc in range(DC):
                    for qc in range(NQ):
                        nc.scalar.copy(
                            dst[mc][:, qc * 512:(qc + 1) * 512], psq[mc * NQ + qc]
                        )
            relmat = rmpool.tile([128, QT], F32, name="relmat")
            for qt in range(QT):
                relcols = rcpool.tile([128, NQ], F32, tag="relcols", name="relcols")
                spss = [
                    pspool.tile([128, 512], F32, tag=f"m{kc}", name="sps")
                    for kc in range(NQ)
                ]
                for d in range(DC):
                    for kc in range(NQ):
                        nc.tensor.matmul(
                            spss[kc],
                            qpt[d][:, qt * 128:(qt + 1) * 128],
                            kpt[d][:, kc * 512:(kc + 1) * 512],
                            start=(d == 0), stop=(d == DC - 1),
                        )
                for kc in range(NQ):
                    scratch = scpool.tile([128, 512], F16, tag="scratch", name="scratch")
                    nc.scalar.activation(
                        scratch, spss[kc], mybir.ActivationFunctionType.Relu,
                        accum_out=relcols[:, kc:kc + 1],
                    )
                nc.vector.tensor_reduce(
                    relmat[:, qt:qt + 1], relcols, axis=mybir.AxisListType.X,
                    op=mybir.AluOpType.add,
                )
            nc.sync.dma_start(
                out=rel.rearrange("(t p) -> p t", p=128), in_=relmat
            )
    nc.compile()
    return nc


def build_l3(S=2048, H=HIDDEN, NHC=NUM_HEADS // N_CORES, HD=HEAD_DIM,
             window=LOCAL_WINDOW):
    """Per-core (attention heads): partial (S, H) = sum_h softmax-attn @ Wo rows."""
    nc = bacc.Bacc("TRN2", target_bir_lowering=False, debug=False)
    KC, NQ, QT, OCC = S // 128, S // 512, S // 128, H // 512
    WT = window // 128  # local window in k-tiles
    qTh = nc.dram_tensor("qTh", [NHC * HD, S], F32R, kind="ExternalInput")
    kTh = nc.dram_tensor("kTh", [NHC * HD, S], F32R, kind="ExternalInput")
    vTh = nc.dram_tensor("vTh", [NHC * HD, S], F32, kind="ExternalInput")
    woh = nc.dram_tensor("woh", [NHC * HD, H], F32R, kind="ExternalInput")
    kidx = nc.dram_tensor("kidx", [S], F16, kind="ExternalInput")
    hivec = nc.dram_tensor("hivec", [S], F16, kind="ExternalInput")
    selv = nc.dram_tensor("selv", [S], F16, kind="ExternalInput")
    onesrow = nc.dram_tensor("onesrow", [128], F32R, kind="ExternalInput")
    part = nc.dram_tensor("part", [S, H], F32, kind="ExternalOutput")

    scale = 1.0 / math.sqrt(HD)
    AF = mybir.ActivationFunctionType
    OP = mybir.AluOpType

    with TileContext(nc) as tc:
        with (
            tc.tile_pool(name="const", bufs=1) as cpool,
            tc.tile_pool(name="qk", bufs=1) as qkpool,
            tc.tile_pool(name="vt", bufs=2) as vtpool,
            tc.tile_pool(name="vh", bufs=1) as vhpool,
            tc.tile_pool(name="vsl", bufs=1) as vslpool,
            tc.tile_pool(name="et", bufs=2) as etpool,
            tc.tile_pool(name="aon", bufs=1) as aopool,
            tc.tile_pool(name="dr", bufs=2) as drpool,
            tc.tile_pool(name="ev", bufs=4) as evpool,
            tc.tile_pool(name="ps", bufs=1, space="PSUM") as pspool,
        ):
            iota = cpool.tile([128, S], F16, name="iota")
            nc.gpsimd.iota(
                iota, pattern=[[1, S]], base=0, channel_multiplier=0,
                allow_small_or_imprecise_dtypes=True,
            )
            ones = cpool.tile([128, 1], F16, name="ones")
            nc.vector.memset(ones, 1.0)
            ident = cpool.tile([128, 128], F32, name="ident")
            make_identity(nc, ident)
            kvec = cpool.tile([128, KC], F16, name="kvec")
            nc.sync.dma_start(out=kvec, in_=kidx.rearrange("(t p) -> p t", p=128))
            hvec = cpool.tile([128, KC], F16, name="hvec")
            nc.sync.dma_start(out=hvec, in_=hivec.rearrange("(t p) -> p t", p=128))
            svec = cpool.tile([128, KC], F16, name="svec")
            nc.sync.dma_start(out=svec, in_=selv.rearrange("(t p) -> p t", p=128))
            svec32 = cpool.tile([128, KC], F32, name="svec32")
            nc.vector.tensor_copy(svec32, svec)
            ones1 = cpool.tile([1, 128], F32R, name="ones1")
            nc.sync.dma_start(out=ones1, in_=onesrow[None, :])

            # head-0 working set first so PE can start early; wo weights last.
            vts0 = vtpool.tile([128, S], F32, tag="vts", name="vts")
            nc.sync.dma_start(out=vts0, in_=vTh[0:HD, :])
            qsb, ksb = [], []
            for h in range(NHC):
                q = qkpool.tile([128, S], F32R, name=f"qsb{h}")
                nc.sync.dma_start(out=q, in_=qTh[h * HD:(h + 1) * HD, :])
                qsb.append(q)
                k = qkpool.tile([128, S], F32R, name=f"ksb{h}")
                nc.sync.dma_start(out=k, in_=kTh[h * HD:(h + 1) * HD, :])
                ksb.append(k)

            aon = [aopool.tile([128, S], F32R, name=f"aon{h}") for h in range(NHC)]
            vhf = [vhpool.tile([128, S], F16, name=f"vhf{h}") for h in range(NHC)]

            for h in range(NHC):
                if h == 0:
                    vts = vts0
                else:
                    vts = vtpool.tile([128, S], F32, tag="vts", name="vts")
                    nc.sync.dma_start(out=vts, in_=vTh[h * HD:(h + 1) * HD, :])
                for kc in range(KC):
                    tp = pspool.tile([128, 128], F32, tag="sc", bufs=3, name="tp")
                    nc.tensor.transpose(tp, vts[:, kc * 128:(kc + 1) * 128], ident)
                    nc.scalar.copy(vhf[h][:, kc * 128:(kc + 1) * 128], tp)
                # v pre-multiplied by the selected mask: beyond-local tiles use
                # it as the stationary operand, making masking free there.
                vsl = vslpool.tile([128, S], F16, name=f"vsl{h}")
                for kc in range(KC):
                    nc.vector.tensor_scalar_mul(
                        vsl[:, kc * 128:(kc + 1) * 128],
                        vhf[h][:, kc * 128:(kc + 1) * 128],
                        svec32[:, kc:kc + 1],
                    )
                # kc-outer: stationary operands (k tile, v tile) reused across
                # the q chunks; av/den accumulate per q chunk across kc.
                avp = [
                    pspool.tile([128, 512], F32, tag=f"av{qc}", bufs=1,
                                name=f"av{qc}")
                    for qc in range(NQ)
                ]
                den128 = pspool.tile([128, 512], F32, tag="den", bufs=1,
                                     name="den128")
                ets = {}
                for kc in range(KC):
                    k0 = kc * 128
                    qcs = [qc for qc in range(NQ) if k0 <= qc * 512 + 511]
                    far = {qc: qc * 512 > k0 + 127 + window for qc in qcs}
                    for qc in qcs:
                        q0 = qc * 512
                        q1 = q0 + 511
                        sps = pspool.tile([128, 512], F32, tag="sc", bufs=3,
                                          name="sps")
                        nc.tensor.matmul(
                            sps, ksb[h][:, kc * 128:(kc + 1) * 128],
                            qsb[h][:, q0:q0 + 512], start=True, stop=True,
                        )
                        et = etpool.tile([128, 512], F16, tag=f"et{qc}",
                                         name=f"et{qc}")
                        ets[qc] = et
                        nc.scalar.activation(et, sps, AF.Exp, scale=scale)
                        if far[qc]:
                            continue  # sel-mask folded into vsl/svec operands
                        if q0 < k0 + 128:
                            # causal: zero where q < k (iota - k < 0)
                            nc.gpsimd.affine_select(
                                out=et, in_=et, compare_op=OP.is_ge, fill=0.0,
                                base=q0 - k0, channel_multiplier=-1,
                                pattern=[[1, 512]],
                            )
                        if q1 > k0 + window:
                            nc.vector.scalar_tensor_tensor(
                                et, iota[:, q0:q0 + 512], hvec[:, kc:kc + 1], et,
                                op0=OP.is_le, op1=OP.mult,
                            )
                    for qc in qcs:
                        lhs_av = vsl if far[qc] else vhf[h]
                        nc.tensor.matmul(
                            avp[qc], lhs_av[:, kc * 128:(kc + 1) * 128], ets[qc],
                            start=(kc == 0), stop=(kc == (qc * 512 + 511) // 128),
                        )
                    for qc in qcs:
                        lhs_den = svec[:, kc:kc + 1] if far[qc] else ones
                        nc.tensor.matmul(
                            den128[32 * qc:32 * qc + 1, :], lhs_den, ets[qc],
                            start=(kc == 0), stop=(kc == (qc * 512 + 511) // 128),
                            tile_position=(0, 32 * qc),
                        )
                # denominators -> reciprocals -> broadcast -> normalize
                for qc in range(NQ):
                    q0 = qc * 512
                    dq = drpool.tile([1, 512], F32, tag=f"dq{qc}", name=f"dq{qc}")
                    nc.scalar.copy(dq, den128[32 * qc:32 * qc + 1, :])
                    rq = drpool.tile([1, 512], F32, tag=f"rq{qc}", name=f"rq{qc}")
                    rs = drpool.tile([1, 512], F32, tag=f"rs{qc}", name=f"rs{qc}")
                    nc.vector.reciprocal_approx_accurate(rq, dq, rs)
                    rcq = drpool.tile([1, 512], F32R, tag=f"rcq{qc}",
                                      name=f"rcq{qc}")
                    nc.vector.tensor_copy(rcq, rq)
                    rb = pspool.tile([128, 512], F32, tag="sc", bufs=3, name="rb")
                    nc.tensor.matmul(rb, ones1, rcq, start=True, stop=True)
                    rbs = drpool.tile([128, 512], F32, tag="rbs", name="rbs")
                    nc.scalar.copy(rbs, rb)
                    nc.vector.scalar_tensor_tensor(
                        aon[h][:, q0:q0 + 512], rbs, 1.0, avp[qc],
                        op0=OP.mult, op1=OP.mult,
                    )
            wsb = []
            for h in range(NHC):
                w = qkpool.tile([128, H], F32R, name=f"wsb{h}")
                nc.sync.dma_start(out=w, in_=woh[h * HD:(h + 1) * HD, :])
                wsb.append(w)
            for qt in range(QT):
                wops = [
                    pspool.tile([128, 512], F32, tag=f"av{oc}", bufs=1,
                                name=f"wops{oc}")
                    for oc in range(OCC)
                ]
                for h in range(NHC):
                    for oc in range(OCC):
                        nc.tensor.matmul(
                            wops[oc], aon[h][:, qt * 128:(qt + 1) * 128],
                            wsb[h][:, oc * 512:(oc + 1) * 512],
                            start=(h == 0), stop=(h == NHC - 1),
                        )
                for oc in range(OCC):
                    ot = evpool.tile([128, 512], F32, tag="ot", name="ot")
                    nc.vector.tensor_copy(ot, wops[oc])
                    nc.sync.dma_start(
                        out=part[qt * 128:(qt + 1) * 128, oc * 512:(oc + 1) * 512],
                        in_=ot,
                    )
    nc.compile()
    return nc


_CACHE = {}


def _get(name, builder, *args):
    key = (name,) + args
    if key not in _CACHE:
        _CACHE[key] = builder(*args)
    return _CACHE[key]


def _run(nc, in_maps):
    res = run_bass_kernel_spmd(
        nc, in_maps, core_ids=list(range(N_CORES)), trace=_TRACE["on"]
    )
    if _TRACE["on"] and res.exec_time_ns is not None:
        _TRACE["exec_ns"].append(res.exec_time_ns)
    return res.results


def kernel(hidden_states, Wq, Wk, Wv, Wo, Wq_ind, Wk_ind, head_weights,
           temperature_param):
    hidden_states = np.asarray(hidden_states, dtype=FP32)
    Wq, Wk, Wv, Wo = (np.asarray(a, dtype=FP32) for a in (Wq, Wk, Wv, Wo))
    Wq_ind = np.asarray(Wq_ind, dtype=FP32)
    Wk_ind = np.asarray(Wk_ind, dtype=FP32)
    head_weights = np.asarray(head_weights, dtype=FP32)
    temp = float(np.asarray(temperature_param))

    B, S, H = hidden_states.shape
    assert B == 1 and H == HIDDEN
    CS = H // N_CORES
    hidT = np.ascontiguousarray(hidden_states[0].T)

    # ---- L1: projections, column-parallel ----
    nc1 = _get("l1", build_l1, S, H, CS)
    in1 = [
        {
            "hidT": hidT,
            "wq": np.ascontiguousarray(Wq[:, c * CS:(c + 1) * CS]),
            "wk": np.ascontiguousarray(Wk[:, c * CS:(c + 1) * CS]),
            "wv": np.ascontiguousarray(Wv[:, c * CS:(c + 1) * CS]),
        }
        for c in range(N_CORES)
    ]
    r1 = _run(nc1, in1)
    qTf = np.concatenate([r["qT"] for r in r1], axis=0)
    kTf = np.concatenate([r["kT"] for r in r1], axis=0)
    vTf = np.concatenate([r["vT"] for r in r1], axis=0)

    # ---- L2: lightning indexer, head-parallel ----
    D = IND_DIM
    nc2 = _get("l2", build_l2, S, H, D)
    in2 = [
        {
            "qT": qTf,
            "kT": kTf,
            "wqi": np.ascontiguousarray(Wq_ind[:, c * D:(c + 1) * D]),
            "wki": np.ascontiguousarray(Wk_ind[:, c * D:(c + 1) * D]),
        }
        for c in range(N_CORES)
    ]
    r2 = _run(nc2, in2)
    rel = np.zeros(S, dtype=np.float64)
    for c in range(N_CORES):
        rel += float(head_weights[c]) * r2[c]["rel"].astype(np.float64)
    # exp(-temp) scaling is monotone; irrelevant for top-k selection.

    k_sel = min(MAX_SELECTED, S)
    top_idx = np.argpartition(-rel, k_sel - 1)[:k_sel]
    selected = np.zeros(S, dtype=bool)
    selected[top_idx] = True

    # ---- L3: masked attention + output projection, head-parallel ----
    BIG = float(2 * S + 1024)
    hi = np.where(selected, BIG, np.arange(S, dtype=np.float64) + LOCAL_WINDOW)
    hi = hi.astype(np.float16)
    kidx = np.arange(S, dtype=np.float16)
    selv = selected.astype(np.float16)
    NHC = NUM_HEADS // N_CORES
    RW = NHC * HEAD_DIM
    nc3 = _get("l3", build_l3, S, H, NHC, HEAD_DIM, LOCAL_WINDOW)
    in3 = [
        {
            "qTh": np.ascontiguousarray(qTf[c * RW:(c + 1) * RW]),
            "kTh": np.ascontiguousarray(kTf[c * RW:(c + 1) * RW]),
            "vTh": np.ascontiguousarray(vTf[c * RW:(c + 1) * RW]),
            "woh": np.ascontiguousarray(Wo[c * RW:(c + 1) * RW]),
            "kidx": kidx,
            "hivec": hi,
            "selv": selv,
            "onesrow": np.ones(128, dtype=np.float32),
        }
        for c in range(N_CORES)
    ]
    r3 = _run(nc3, in3)
    out = r3[0]["part"]
    for c in range(1, N_CORES):
        out = out + r3[c]["part"]
    return out.reshape(B, S, H).astype(np.float32)



# revision 6
# speedup vs baseline: 1.2637x; 1.2637x over previous
"""DeepSeek sparse attention on 8 Trainium2 NeuronCores (Bass/Tile).

Two SPMD launches:

  A (projections + lightning indexer, column/head-parallel): core c computes
     the 256-column slice (= its 2 attention heads) of q/k/v as fp16 (256,S)
     from f32r hidden^T resident in SBUF, PLUS the indexer-head-c projections
     qp_c/kp_c (256,S) using HOST-FUSED weights Wq@Wq_ind / Wk@Wk_ind (f32r,
     full precision — the top-k selection needs ~1e-5 relative accuracy), and
     rel_c[t] = sum_k relu(qp_c[t] . kp_c[k]) via PE + one relu-accumulate
     ACT pass per 128-token tile. Indexer passes run first so they overlap
     the hidden DMA window; scores interleave with the q/k/v passes.
  host: rel = sum_c w_c rel_c; top-1024 -> selected mask; hi/sel vectors.
  B (attention, head-parallel): core c feeds ITS OWN fp16 q/k/v slices from
     launch A straight back (no concat), computes causal/local/selected
     masked softmax attention for heads 2c,2c+1 and the partial output
     projection (S,H) in fp16; host sums the 8 fp16 partials in fp32.

All matmuls f32r or fp16 (1 PE cycle/row at N=512). fp16 everywhere in B
(calibrated: bf16 q/k/v + 16-bit partial store => 2.9e-3 rel err; fp16 is
strictly tighter; indexer stays f32r — one top-k swap costs ~1.5e-2).
"""

import math

import numpy as np

import concourse.bass as bass
import concourse.mybir as mybir
from concourse import bacc
from concourse.tile import TileContext
from concourse.masks import make_identity
from concourse.bass_utils import run_bass_kernel_spmd

# Problem constants (hardcoded per contract)
HIDDEN = 2048
NUM_HEADS = 16
HEAD_DIM = 128
NUM_IND_HEADS = 8
IND_DIM = HIDDEN // NUM_IND_HEADS  # 256
MAX_SELECTED = 1024
LOCAL_WINDOW = 512
N_CORES = 8

F32 = mybir.dt.float32
F32R = mybir.dt.float32r
F16 = mybir.dt.float16
FP32 = np.float32

_TRACE = {"on": False, "exec_ns": []}


def build_a(S=2048, H=HIDDEN, CS=HIDDEN // N_CORES):
    """Per-core: q/k/v column slices (CS,S) fp16 + indexer rel_c (S) f32."""
    nc = bacc.Bacc("TRN2", target_bir_lowering=False, debug=False)
    HT, MC, NQ, QT = H // 128, CS // 128, S // 512, S // 128
    hidT = nc.dram_tensor("hidT", [H, S], F32R, kind="ExternalInput")
    wq = nc.dram_tensor("wq", [H, CS], F32R, kind="ExternalInput")
    wk = nc.dram_tensor("wk", [H, CS], F32R, kind="ExternalInput")
    wv = nc.dram_tensor("wv", [H, CS], F32R, kind="ExternalInput")
    wfq = nc.dram_tensor("wfq", [H, CS], F32R, kind="ExternalInput")
    wfk = nc.dram_tensor("wfk", [H, CS], F32R, kind="ExternalInput")
    qT = nc.dram_tensor("qT", [CS, S], F16, kind="ExternalOutput")
    kT = nc.dram_tensor("kT", [CS, S], F16, kind="ExternalOutput")
    vT = nc.dram_tensor("vT", [CS, S], F16, kind="ExternalOutput")
    rel = nc.dram_tensor("rel", [S], F32, kind="ExternalOutput")

    G = 8          # hidden chunks
    TG = HT // G   # strips per chunk

    with TileContext(nc) as tc:
        with (
            tc.tile_pool(name="hid", bufs=1) as hpool,
            tc.tile_pool(name="wt", bufs=2) as wpool,
            tc.tile_pool(name="proj", bufs=1) as ppool,
            tc.tile_pool(name="st", bufs=2) as stpool,
            tc.tile_pool(name="scr", bufs=1) as scrpool,
            tc.tile_pool(name="rm", bufs=1) as rmpool,
            tc.tile_pool(name="ps", bufs=1, space="PSUM") as pspool,
        ):
            # ---- input DMAs: wfq (2 halves for an earlier first matmul),
            # wfk, then hidden chunks. wq/wk/wv are issued later, at the
            # program points where their weight-pool slot is freed.
            def load_w(wdram, halves=1):
                wr = wpool.tile([128, HT * CS], F32R, tag="w", name="w")
                hh = HT // halves
                for i in range(halves):
                    nc.sync.dma_start(
                        out=wr[:, i * hh * CS:(i + 1) * hh * CS].rearrange(
                            "p (t c) -> p t c", t=hh
                        ),
                        in_=wdram[i * hh * 128:(i + 1) * hh * 128, :].rearrange(
                            "(t p) c -> p t c", p=128
                        ),
                    )
                return wr

            wfq_t = load_w(wfq, halves=2)
            wfk_t = load_w(wfk)

            hidc = []
            for g in range(G):
                hc = hpool.tile([128, TG * S], F32R, name=f"hidc{g}")
                nc.sync.dma_start(
                    out=hc.rearrange("p (t s) -> p t s", t=TG),
                    in_=hidT[g * TG * 128:(g + 1) * TG * 128, :].rearrange(
                        "(t p) s -> p t s", p=128
                    ),
                )
                hidc.append(hc)

            # resident f32r indexer projections qp^T/kp^T (2 x 128 x S each)
            qpt = [ppool.tile([128, S], F32R, name=f"qpt{m}") for m in range(MC)]
            kpt = [ppool.tile([128, S], F32R, name=f"kpt{m}") for m in range(MC)]

            # psum regions: 4 banks for projection passes ("pj"), and one
            # [128, S] region ("scr") that serves double duty: the kp passes
            # accumulate in its 512-slices, and the indexer-score tiles use
            # it whole.
            def pj_psums():
                return [
                    pspool.tile([128, 512], F32, tag=f"pj{i}", name=f"pj{i}")
                    for i in range(NQ)
                ]

            def scr_psum():
                return pspool.tile([128, S], F32, tag="scr", name="scr")

            relmat = rmpool.tile([128, QT], F32, name="relmat")
            scratch = scrpool.tile([128, S], F16, name="scratch")

            score_state = {"next": 0}

            def emit_score_qt():
                """Indexer scores for one 128-token tile: 8 matmuls into the
                scr psum region + one relu-accumulate -> relmat column."""
                qt = score_state["next"]
                if qt >= QT:
                    return False
                score_state["next"] += 1
                sps = scr_psum()
                for d in range(MC):
                    for kc in range(NQ):
                        nc.tensor.matmul(
                            sps[:, kc * 512:(kc + 1) * 512],
                            qpt[d][:, qt * 128:(qt + 1) * 128],
                            kpt[d][:, kc * 512:(kc + 1) * 512],
                            start=(d == 0), stop=(d == MC - 1),
                        )
                nc.scalar.activation(
                    scratch, sps, mybir.ActivationFunctionType.Relu,
                    accum_out=relmat[:, qt:qt + 1],
                )
                return True

            def sl(psums, qc):
                return (psums[qc] if isinstance(psums, list)
                        else psums[:, qc * 512:(qc + 1) * 512])

            def proj_passes(groups, score_slots=()):
                """Interleaved m-tile passes: each group = (wtile, mc, psums,
                finish). Strips advance together so every group progresses
                chunk-by-chunk behind the hidden DMA."""
                for t in range(HT):
                    for wtile, mc, psums, _ in groups:
                        lhsT = wtile[:, t * CS + mc * 128:
                                     t * CS + mc * 128 + 128]
                        rhs = hidc[t // TG]
                        tl = t % TG
                        for qc in range(NQ):
                            nc.tensor.matmul(
                                sl(psums, qc), lhsT,
                                rhs[:, tl * S + qc * 512:
                                    tl * S + qc * 512 + 512],
                                start=(t == 0), stop=(t == HT - 1),
                            )
                    if t in score_slots:
                        emit_score_qt()
                for _, _, _, finish in groups:
                    finish()

            def copy_to(dst, psums):
                def fin():
                    for qc in range(NQ):
                        nc.vector.tensor_copy(
                            dst[:, qc * 512:(qc + 1) * 512], sl(psums, qc)
                        )
                return fin

            # ---- indexer projection passes; the m0 pair overlaps the
            # hidden-DMA window (qp-m0 on pj banks, kp-m0 on scr region).
            pj = pj_psums()
            sc = scr_psum()
            proj_passes([
                (wfq_t, 0, pj, copy_to(qpt[0], pj)),
                (wfk_t, 0, sc, copy_to(kpt[0], sc)),
            ])
            pj = pj_psums()
            sc = scr_psum()
            proj_passes([
                (wfq_t, 1, pj, copy_to(qpt[1], pj)),
                (wfk_t, 1, sc, copy_to(kpt[1], sc)),
            ])
            # wfq/wfk slots free now -> issue wq/wk loads
            wq_t = load_w(wq)
            wk_t = load_w(wk)

            # ---- q/k/v passes (fp16 out) with indexer scores interleaved
            def store_pass(wtile, mc, odram):
                psums = pj_psums()
                stage = stpool.tile([128, S], F16, tag="st", name="st")

                def fin():
                    for qc in range(NQ):
                        nc.vector.tensor_copy(
                            stage[:, qc * 512:(qc + 1) * 512], psums[qc]
                        )
                    nc.sync.dma_start(
                        out=odram[mc * 128:(mc + 1) * 128, :], in_=stage
                    )
                proj_passes([(wtile, mc, psums, fin)],
                            score_slots=(2, 7, 12))

            store_pass(wq_t, 0, qT)
            store_pass(wq_t, 1, qT)
            wv_t = load_w(wv)
            store_pass(wk_t, 0, kT)
            store_pass(wk_t, 1, kT)
            store_pass(wv_t, 0, vT)
            store_pass(wv_t, 1, vT)
            while emit_score_qt():
                pass

            nc.sync.dma_start(
                out=rel.rearrange("(t p) -> p t", p=128), in_=relmat
            )
    nc.compile()
    return nc


def build_b(S=2048, H=HIDDEN, NHC=NUM_HEADS // N_CORES, HD=HEAD_DIM,
            window=LOCAL_WINDOW):
    """Per-core attention for 2 heads + fp16 partial output projection."""
    nc = bacc.Bacc("TRN2", target_bir_lowering=False, debug=False)
    KC, NQ, QT, OCC = S // 128, S // 512, S // 128, H // 512
    qTh = nc.dram_tensor("qTh", [NHC * HD, S], F16, kind="ExternalInput")
    kTh = nc.dram_tensor("kTh", [NHC * HD, S], F16, kind="ExternalInput")
    vTh = nc.dram_tensor("vTh", [NHC * HD, S], F16, kind="ExternalInput")
    woh = nc.dram_tensor("woh", [NHC * HD, H], F16, kind="ExternalInput")
    kidx = nc.dram_tensor("kidx", [S], F16, kind="ExternalInput")
    hivec = nc.dram_tensor("hivec", [S], F16, kind="ExternalInput")
    selv = nc.dram_tensor("selv", [S], F16, kind="ExternalInput")
    onesrow = nc.dram_tensor("onesrow", [128], F32R, kind="ExternalInput")
    part = nc.dram_tensor("part", [S, H], F16, kind="ExternalOutput")

    scale = 1.0 / math.sqrt(HD)
    AF = mybir.ActivationFunctionType
    OP = mybir.AluOpType

    with TileContext(nc) as tc:
        with (
            tc.tile_pool(name="const", bufs=1) as cpool,
            tc.tile_pool(name="qk", bufs=1) as qkpool,
            tc.tile_pool(name="vt", bufs=2) as vtpool,
            tc.tile_pool(name="vh", bufs=1) as vhpool,
            tc.tile_pool(name="et", bufs=3) as etpool,
            tc.tile_pool(name="aon", bufs=1) as aopool,
            tc.tile_pool(name="dr", bufs=2) as drpool,
            tc.tile_pool(name="ost", bufs=2) as ostpool,
            tc.tile_pool(name="ps", bufs=1, space="PSUM") as pspool,
        ):
            # q/k for both heads first so PE can start scoring early
            qsb, ksb = [], []
            for h in range(NHC):
                k = qkpool.tile([128, S], F16, name=f"ksb{h}")
                nc.sync.dma_start(out=k, in_=kTh[h * HD:(h + 1) * HD, :])
                ksb.append(k)
                q = qkpool.tile([128, S], F16, name=f"qsb{h}")
                nc.sync.dma_start(out=q, in_=qTh[h * HD:(h + 1) * HD, :])
                qsb.append(q)
            vts0 = vtpool.tile([128, S], F16, tag="vts", name="vts")
            nc.sync.dma_start(out=vts0, in_=vTh[0:HD, :])

            kvec = cpool.tile([128, KC], F16, name="kvec")
            nc.sync.dma_start(out=kvec, in_=kidx.rearrange("(t p) -> p t", p=128))
            hvec = cpool.tile([128, KC], F16, name="hvec")
            nc.sync.dma_start(out=hvec, in_=hivec.rearrange("(t p) -> p t", p=128))
            svec = cpool.tile([128, KC], F16, name="svec")
            nc.sync.dma_start(out=svec, in_=selv.rearrange("(t p) -> p t", p=128))
            ones1 = cpool.tile([1, 128], F32R, name="ones1")
            nc.sync.dma_start(out=ones1, in_=onesrow[None, :])

            wsb = []
            for h in range(NHC):
                w = qkpool.tile([128, H], F16, name=f"wsb{h}")
                nc.sync.dma_start(out=w, in_=woh[h * HD:(h + 1) * HD, :])
                wsb.append(w)

            svec32 = cpool.tile([128, KC], F32, name="svec32")
            nc.vector.tensor_copy(svec32, svec)
            ones = cpool.tile([128, 1], F16, name="ones")
            nc.vector.memset(ones, 1.0)
            ident = cpool.tile([128, 128], F16, name="ident")
            make_identity(nc, ident)
            iota = cpool.tile([128, S], F16, name="iota")
            nc.gpsimd.iota(
                iota, pattern=[[1, S]], base=0, channel_multiplier=0,
                allow_small_or_imprecise_dtypes=True,
            )

            aon = [aopool.tile([128, S], F16, name=f"aon{h}") for h in range(NHC)]
            vhf = [vhpool.tile([128, S], F16, name=f"vhf{h}") for h in range(NHC)]
            vsl = [vhpool.tile([128, S], F16, name=f"vsl{h}") for h in range(NHC)]

            def normalize(h, qc, avp, den):
                q0 = qc * 512
                dq = drpool.tile([1, 512], F32, tag="dq", name="dq")
                nc.scalar.copy(dq, den[0:1, :])
                rq = drpool.tile([1, 512], F32, tag="rq", name="rq")
                rs = drpool.tile([1, 512], F32, tag="rs", name="rs")
                nc.vector.reciprocal_approx_accurate(rq, dq, rs)
                rcq = drpool.tile([1, 512], F32R, tag="rcq", name="rcq")
                nc.vector.tensor_copy(rcq, rq)
                rb = pspool.tile([128, 512], F32, tag="sc", bufs=2, name="rb")
                nc.tensor.matmul(rb, ones1, rcq, start=True, stop=True)
                rbs = drpool.tile([128, 512], F32, tag="rbs", name="rbs")
                nc.scalar.copy(rbs, rb)
                nc.vector.scalar_tensor_tensor(
                    aon[h][:, q0:q0 + 512], rbs, 1.0, avp,
                    op0=OP.mult, op1=OP.mult,
                )

            def outproj(qc):
                """Output projection for the 4 query tiles of chunk qc,
                accumulating both heads; fp16 stage -> one DMA per tile."""
                for qt in range(qc * NQ, qc * NQ + NQ):
                    ostage = ostpool.tile([128, H], F16, tag="ost", name="ost")
                    for oc in range(OCC):
                        wop = pspool.tile([128, 512], F32, tag="wo", bufs=2,
                                          name="wo")
                        for h in range(NHC):
                            nc.tensor.matmul(
                                wop, aon[h][:, qt * 128:(qt + 1) * 128],
                                wsb[h][:, oc * 512:(oc + 1) * 512],
                                start=(h == 0), stop=(h == NHC - 1),
                            )
                        nc.vector.tensor_copy(
                            ostage[:, oc * 512:(oc + 1) * 512], wop
                        )
                    nc.sync.dma_start(
                        out=part[qt * 128:(qt + 1) * 128, :], in_=ostage
                    )

            for h in range(NHC):
                if h == 0:
                    vts = vts0
                else:
                    vts = vtpool.tile([128, S], F16, tag="vts", name="vts")
                    nc.sync.dma_start(out=vts, in_=vTh[h * HD:(h + 1) * HD, :])
                # v tiles transposed to (k, hd) layout; vsl = v * selected
                for kc in range(KC):
                    tp = pspool.tile([128, 128], F16, tag="tp", bufs=1, name="tp")
                    nc.tensor.transpose(tp, vts[:, kc * 128:(kc + 1) * 128], ident)
                    nc.scalar.copy(vhf[h][:, kc * 128:(kc + 1) * 128], tp)
                    nc.vector.tensor_scalar_mul(
                        vsl[h][:, kc * 128:(kc + 1) * 128],
                        vhf[h][:, kc * 128:(kc + 1) * 128],
                        svec32[:, kc:kc + 1],
                    )

                for qc in range(NQ):
                    q0 = qc * 512
                    kcm = (q0 + 511) // 128  # last causal k-tile
                    avp = pspool.tile([128, 512], F32, tag="av", bufs=2,
                                      name="avp")
                    den = pspool.tile([128, 512], F32, tag="den", bufs=1,
                                      name="den")
                    for kc in range(kcm + 1):
                        k0 = kc * 128
                        far = q0 > k0 + 127 + window
                        sps = pspool.tile([128, 512], F32, tag="sc", bufs=2,
                                          name="sps")
                        nc.tensor.matmul(
                            sps, ksb[h][:, k0:k0 + 128], qsb[h][:, q0:q0 + 512],
                            start=True, stop=True,
                        )
                        et = etpool.tile([128, 512], F16, tag="et", name="et")
                        nc.scalar.activation(et, sps, AF.Exp, scale=scale)
                        if q0 < k0 + 128:
                            nc.gpsimd.affine_select(
                                out=et, in_=et, compare_op=OP.is_ge, fill=0.0,
                                base=q0 - k0, channel_multiplier=-1,
                                pattern=[[1, 512]],
                            )
                        elif not far and q0 + 511 > k0 + window:
                            nc.vector.scalar_tensor_tensor(
                                et, iota[:, q0:q0 + 512], hvec[:, kc:kc + 1], et,
                                op0=OP.is_le, op1=OP.mult,
                            )
                        nc.tensor.matmul(
                            avp, (vsl if far else vhf)[h][:, k0:k0 + 128], et,
                            start=(kc == 0), stop=(kc == kcm),
                        )
                        nc.tensor.matmul(
                            den[0:1, :],
                            svec[:, kc:kc + 1] if far else ones, et,
                            start=(kc == 0), stop=(kc == kcm),
                            tile_position=(0, 0),
                        )
                    normalize(h, qc, avp, den)
                    if h == NHC - 1:
                        outproj(qc)
    nc.compile()
    return nc


_CACHE = {}


def _get(name, builder, *args):
    key = (name,) + args
    if key not in _CACHE:
        _CACHE[key] = builder(*args)
    return _CACHE[key]


def _run(nc, in_maps):
    res = run_bass_kernel_spmd(
        nc, in_maps, core_ids=list(range(N_CORES)), trace=_TRACE["on"]
    )
    if _TRACE["on"] and res.exec_time_ns is not None:
        _TRACE["exec_ns"].append(res.exec_time_ns)
    return res.results


def kernel(hidden_states, Wq, Wk, Wv, Wo, Wq_ind, Wk_ind, head_weights,
           temperature_param):
    hidden_states = np.asarray(hidden_states, dtype=FP32)
    Wq, Wk, Wv, Wo = (np.asarray(a, dtype=FP32) for a in (Wq, Wk, Wv, Wo))
    Wq_ind = np.asarray(Wq_ind, dtype=FP32)
    Wk_ind = np.asarray(Wk_ind, dtype=FP32)
    head_weights = np.asarray(head_weights, dtype=FP32)

    B, S, H = hidden_states.shape
    assert B == 1 and H == HIDDEN
    CS = H // N_CORES
    hidT = np.ascontiguousarray(hidden_states[0].T)
    # host-fused indexer weights (fp64 for exactness)
    Wfq = (Wq.astype(np.float64) @ Wq_ind.astype(np.float64)).astype(FP32)
    Wfk = (Wk.astype(np.float64) @ Wk_ind.astype(np.float64)).astype(FP32)

    # ---- launch A ----
    nca = _get("a", build_a, S, H, CS)
    ina = [
        {
            "hidT": hidT,
            "wq": np.ascontiguousarray(Wq[:, c * CS:(c + 1) * CS]),
            "wk": np.ascontiguousarray(Wk[:, c * CS:(c + 1) * CS]),
            "wv": np.ascontiguousarray(Wv[:, c * CS:(c + 1) * CS]),
            "wfq": np.ascontiguousarray(Wfq[:, c * CS:(c + 1) * CS]),
            "wfk": np.ascontiguousarray(Wfk[:, c * CS:(c + 1) * CS]),
        }
        for c in range(N_CORES)
    ]
    ra = _run(nca, ina)

    rel = np.zeros(S, dtype=np.float64)
    for c in range(N_CORES):
        rel += float(head_weights[c]) * ra[c]["rel"].astype(np.float64)
    # exp(-temp) scaling is monotone; irrelevant for top-k selection.

    k_sel = min(MAX_SELECTED, S)
    top_idx = np.argpartition(-rel, k_sel - 1)[:k_sel]
    selected = np.zeros(S, dtype=bool)
    selected[top_idx] = True

    # ---- launch B ----
    BIG = float(2 * S + 1024)
    hi = np.where(selected, BIG, np.arange(S, dtype=np.float64) + LOCAL_WINDOW)
    inb = [
        {
            "qTh": ra[c]["qT"],
            "kTh": ra[c]["kT"],
            "vTh": ra[c]["vT"],
            "woh": Wo[c * CS:(c + 1) * CS].astype(np.float16),
            "kidx": np.arange(S, dtype=np.float16),
            "hivec": hi.astype(np.float16),
            "selv": selected.astype(np.float16),
            "onesrow": np.ones(128, dtype=np.float32),
        }
        for c in range(N_CORES)
    ]
    ncb = _get("b", build_b, S, H, NUM_HEADS // N_CORES, HEAD_DIM, LOCAL_WINDOW)
    rb = _run(ncb, inb)
    out = np.zeros((S, H), dtype=np.float32)
    for c in range(N_CORES):
        out += rb[c]["part"].astype(np.float32)
    return out.reshape(B, S, H)


# revision 16
# speedup vs baseline: 1.3380x; 1.0588x over previous
"""DeepSeek sparse attention on 8 Trainium2 NeuronCores (Bass/Tile).

Two SPMD launches:

  A (projections + lightning indexer, column/head-parallel): core c computes
     the 256-column slice (= its 2 attention heads) of q/k/v as fp16 (256,S)
     from f32r hidden^T resident in SBUF, PLUS the indexer-head-c projections
     qp_c/kp_c (256,S) using HOST-FUSED weights Wq@Wq_ind / Wk@Wk_ind (f32r,
     full precision — the top-k selection needs ~1e-5 relative accuracy), and
     rel_c[t] = sum_k relu(qp_c[t] . kp_c[k]) via PE + one relu-accumulate
     ACT pass per 128-token tile. Indexer passes run first so they overlap
     the hidden DMA window; scores interleave with the q/k/v passes.
  host: rel = sum_c w_c rel_c; top-1024 -> selected mask; hi/sel vectors.
  B (attention, head-parallel): core c feeds ITS OWN fp16 q/k/v slices from
     launch A straight back (no concat), computes causal/local/selected
     masked softmax attention for heads 2c,2c+1 and the partial output
     projection (S,H) in fp16; host sums the 8 fp16 partials in fp32.

All matmuls f32r or fp16 (1 PE cycle/row at N=512). fp16 everywhere in B
(calibrated: bf16 q/k/v + 16-bit partial store => 2.9e-3 rel err; fp16 is
strictly tighter; indexer stays f32r — one top-k swap costs ~1.5e-2).
"""

import math

import numpy as np

import concourse.bass as bass
import concourse.mybir as mybir
from concourse import bacc
from concourse.tile import TileContext
from concourse.masks import make_identity
from concourse.bass_utils import run_bass_kernel_spmd

# Problem constants (hardcoded per contract)
HIDDEN = 2048
NUM_HEADS = 16
HEAD_DIM = 128
NUM_IND_HEADS = 8
IND_DIM = HIDDEN // NUM_IND_HEADS  # 256
MAX_SELECTED = 1024
LOCAL_WINDOW = 512
N_CORES = 8

F32 = mybir.dt.float32
F32R = mybir.dt.float32r
F16 = mybir.dt.float16
FP32 = np.float32

_TRACE = {"on": False, "exec_ns": []}


def build_a(S=2048, H=HIDDEN, CS=HIDDEN // N_CORES):
    """Per-core: q/k/v column slices (CS,S) fp16 + indexer rel_c (S) f32."""
    nc = bacc.Bacc("TRN2", target_bir_lowering=False, debug=False)
    HT, MC, NQ, QT = H // 128, CS // 128, S // 512, S // 128
    hidT = nc.dram_tensor("hidT", [H, S], F32R, kind="ExternalInput")
    wq = nc.dram_tensor("wq", [H, CS], F32R, kind="ExternalInput")
    wk = nc.dram_tensor("wk", [H, CS], F32R, kind="ExternalInput")
    wv = nc.dram_tensor("wv", [H, CS], F32R, kind="ExternalInput")
    wfq = nc.dram_tensor("wfq", [H, CS], F32R, kind="ExternalInput")
    wfk = nc.dram_tensor("wfk", [H, CS], F32R, kind="ExternalInput")
    qT = nc.dram_tensor("qT", [CS, S], F16, kind="ExternalOutput")
    kT = nc.dram_tensor("kT", [CS, S], F16, kind="ExternalOutput")
    vT = nc.dram_tensor("vT", [CS, S], F16, kind="ExternalOutput")
    rel = nc.dram_tensor("rel", [S], F32, kind="ExternalOutput")

    G = 8          # hidden chunks
    TG = HT // G   # strips per chunk

    with TileContext(nc) as tc:
        with (
            tc.tile_pool(name="hid", bufs=1) as hpool,
            tc.tile_pool(name="wt", bufs=2) as wpool,
            tc.tile_pool(name="proj", bufs=1) as ppool,
            tc.tile_pool(name="st", bufs=2) as stpool,
            tc.tile_pool(name="scr", bufs=1) as scrpool,
            tc.tile_pool(name="rm", bufs=1) as rmpool,
            tc.tile_pool(name="ps", bufs=1, space="PSUM") as pspool,
        ):
            # ---- input DMAs. Order matters: the first matmul needs the
            # first half of wfq plus hidden chunk 0, so those go first; wfk
            # comes after the hidden chunks (first needed ~50us in); wq/wk/wv
            # are issued later, at the program points where their weight-pool
            # slot is freed (avoids WAR stalls on the slot).
            def load_w(wdram, dt=F32R, halves=1):
                wr = wpool.tile([128, HT * CS], dt, tag="w", name="w")
                hh = HT // halves
                for i in range(halves):
                    nc.sync.dma_start(
                        out=wr[:, i * hh * CS:(i + 1) * hh * CS].rearrange(
                            "p (t c) -> p t c", t=hh
                        ),
                        in_=wdram[i * hh * 128:(i + 1) * hh * 128, :].rearrange(
                            "(t p) c -> p t c", p=128
                        ),
                    )
                return wr

            hidc = [hpool.tile([128, TG * S], F32R, name=f"hidc{g}")
                    for g in range(G)]

            def load_hid(g):
                nc.sync.dma_start(
                    out=hidc[g].rearrange("p (t s) -> p t s", t=TG),
                    in_=hidT[g * TG * 128:(g + 1) * TG * 128, :].rearrange(
                        "(t p) s -> p t s", p=128
                    ),
                )

            wfq_t = wpool.tile([128, HT * CS], F32R, tag="w", name="w")
            HH = HT // 2
            nc.sync.dma_start(
                out=wfq_t[:, :HH * CS].rearrange("p (t c) -> p t c", t=HH),
                in_=wfq[:HH * 128, :].rearrange("(t p) c -> p t c", p=128),
            )
            load_hid(0)
            nc.sync.dma_start(
                out=wfq_t[:, HH * CS:].rearrange("p (t c) -> p t c", t=HH),
                in_=wfq[HH * 128:, :].rearrange("(t p) c -> p t c", p=128),
            )
            for g in range(1, G):
                load_hid(g)
            wfk_t = load_w(wfk)

            # resident f32r indexer projections qp^T/kp^T (2 x 128 x S each)
            qpt = [ppool.tile([128, S], F32R, name=f"qpt{m}") for m in range(MC)]
            kpt = [ppool.tile([128, S], F32R, name=f"kpt{m}") for m in range(MC)]

            # psum regions: 4 banks for projection passes ("pj"), and one
            # [128, S] region ("scr") that serves double duty: the kp passes
            # accumulate in its 512-slices, and the indexer-score tiles use
            # it whole.
            def pj_psums():
                return [
                    pspool.tile([128, 512], F32, tag=f"pj{i}", name=f"pj{i}")
                    for i in range(NQ)
                ]

            def scr_psum():
                return pspool.tile([128, S], F32, tag="scr", name="scr")

            relmat = rmpool.tile([128, QT], F32, name="relmat")
            scratch = scrpool.tile([128, S], F16, name="scratch")

            score_state = {"next": 0}

            def emit_score_qt():
                """Indexer scores for one 128-token tile: 8 matmuls into the
                scr psum region + one relu-accumulate -> relmat column."""
                qt = score_state["next"]
                if qt >= QT:
                    return False
                score_state["next"] += 1
                sps = scr_psum()
                for d in range(MC):
                    for kc in range(NQ):
                        nc.tensor.matmul(
                            sps[:, kc * 512:(kc + 1) * 512],
                            qpt[d][:, qt * 128:(qt + 1) * 128],
                            kpt[d][:, kc * 512:(kc + 1) * 512],
                            start=(d == 0), stop=(d == MC - 1),
                        )
                nc.scalar.activation(
                    scratch, sps, mybir.ActivationFunctionType.Relu,
                    accum_out=relmat[:, qt:qt + 1],
                )
                return True

            def sl(psums, qc):
                return (psums[qc] if isinstance(psums, list)
                        else psums[:, qc * 512:(qc + 1) * 512])

            def proj_passes(groups, score_slots=()):
                """Interleaved m-tile passes: each group = (wtile, mc, psums,
                finish). Strips advance together so every group progresses
                chunk-by-chunk behind the hidden DMA."""
                for t in range(HT):
                    for wtile, mc, psums, _ in groups:
                        lhsT = wtile[:, t * CS + mc * 128:
                                     t * CS + mc * 128 + 128]
                        rhs = hidc[t // TG]
                        tl = t % TG
                        for qc in range(NQ):
                            nc.tensor.matmul(
                                sl(psums, qc), lhsT,
                                rhs[:, tl * S + qc * 512:
                                    tl * S + qc * 512 + 512],
                                start=(t == 0), stop=(t == HT - 1),
                            )
                    if t in score_slots:
                        emit_score_qt()
                for _, _, _, finish in groups:
                    finish()

            def copy_to(dst, psums):
                def fin():
                    for qc in range(NQ):
                        eng = nc.vector if qc % 2 == 0 else nc.scalar
                        if eng is nc.vector:
                            nc.vector.tensor_copy(
                                dst[:, qc * 512:(qc + 1) * 512], sl(psums, qc)
                            )
                        else:
                            nc.scalar.copy(
                                dst[:, qc * 512:(qc + 1) * 512], sl(psums, qc)
                            )
                return fin

            # ---- indexer projection passes; the qp-m0/m1 pair overlaps the
            # hidden-DMA window (both read wfq, which arrives first).
            pj = pj_psums()
            sc = scr_psum()
            proj_passes([
                (wfq_t, 0, pj, copy_to(qpt[0], pj)),
                (wfq_t, 1, sc, copy_to(qpt[1], sc)),
            ])
            # wfq slot free now -> issue wq load
            wq_t = load_w(wq)
            pj = pj_psums()
            sc = scr_psum()
            proj_passes([
                (wfk_t, 0, pj, copy_to(kpt[0], pj)),
                (wfk_t, 1, sc, copy_to(kpt[1], sc)),
            ])
            wk_t = load_w(wk)

            # ---- q/k/v passes (fp16 weights) with indexer scores interleaved
            def store_pass(wtile, mc, odram):
                psums = pj_psums()
                stage = stpool.tile([128, S], F16, tag="st", name="st")

                def fin():
                    for qc in range(NQ):
                        if qc % 2 == 0:
                            nc.vector.tensor_copy(
                                stage[:, qc * 512:(qc + 1) * 512], psums[qc]
                            )
                        else:
                            nc.scalar.copy(
                                stage[:, qc * 512:(qc + 1) * 512], psums[qc]
                            )
                    nc.sync.dma_start(
                        out=odram[mc * 128:(mc + 1) * 128, :], in_=stage
                    )
                proj_passes([(wtile, mc, psums, fin)],
                            score_slots=(1, 6, 11))

            store_pass(wq_t, 0, qT)
            store_pass(wq_t, 1, qT)
            wv_t = load_w(wv)
            store_pass(wk_t, 0, kT)
            store_pass(wk_t, 1, kT)
            store_pass(wv_t, 0, vT)
            store_pass(wv_t, 1, vT)
            while emit_score_qt():
                pass

            nc.sync.dma_start(
                out=rel.rearrange("(t p) -> p t", p=128), in_=relmat
            )
    nc.compile()
    return nc


def build_b(S=2048, H=HIDDEN, NHC=NUM_HEADS // N_CORES, HD=HEAD_DIM,
            window=LOCAL_WINDOW):
    """Per-core attention for 2 heads + fp16 partial output projection."""
    nc = bacc.Bacc("TRN2", target_bir_lowering=False, debug=False)
    KC, NQ, QT, OCC = S // 128, S // 512, S // 128, H // 512
    qTh = nc.dram_tensor("qTh", [NHC * HD, S], F16, kind="ExternalInput")
    kTh = nc.dram_tensor("kTh", [NHC * HD, S], F16, kind="ExternalInput")
    vTh = nc.dram_tensor("vTh", [NHC * HD, S], F16, kind="ExternalInput")
    woh = nc.dram_tensor("woh", [NHC * HD, H], F16, kind="ExternalInput")
    hivec = nc.dram_tensor("hivec", [S], F16, kind="ExternalInput")
    selv = nc.dram_tensor("selv", [S], F16, kind="ExternalInput")
    onesrow = nc.dram_tensor("onesrow", [128], F32R, kind="ExternalInput")
    part = nc.dram_tensor("part", [S, H], F16, kind="ExternalOutput")

    scale = 1.0 / math.sqrt(HD)
    AF = mybir.ActivationFunctionType
    OP = mybir.AluOpType

    with TileContext(nc) as tc:
        with (
            tc.tile_pool(name="const", bufs=1) as cpool,
            tc.tile_pool(name="qk", bufs=1) as qkpool,
            tc.tile_pool(name="vt", bufs=2) as vtpool,
            tc.tile_pool(name="vh", bufs=1) as vhpool,
            tc.tile_pool(name="et", bufs=3) as etpool,
            tc.tile_pool(name="aon", bufs=1) as aopool,
            tc.tile_pool(name="dr", bufs=2) as drpool,
            tc.tile_pool(name="ost", bufs=2) as ostpool,
            tc.tile_pool(name="ps", bufs=1, space="PSUM") as pspool,
        ):
            # v first (transposes are at the head of the PE queue), then q/k
            # of head 0 so scoring starts right behind the transposes.
            vts0 = vtpool.tile([128, S], F16, tag="vts", name="vts")
            nc.sync.dma_start(out=vts0, in_=vTh[0:HD, :])
            qsb, ksb = [], []
            for h in range(NHC):
                k = qkpool.tile([128, S], F16, name=f"ksb{h}")
                nc.sync.dma_start(out=k, in_=kTh[h * HD:(h + 1) * HD, :])
                ksb.append(k)
                q = qkpool.tile([128, S], F16, name=f"qsb{h}")
                nc.sync.dma_start(out=q, in_=qTh[h * HD:(h + 1) * HD, :])
                qsb.append(q)

            hvec = cpool.tile([128, KC], F16, name="hvec")
            nc.sync.dma_start(out=hvec, in_=hivec.rearrange("(t p) -> p t", p=128))
            svec = cpool.tile([128, KC], F16, name="svec")
            nc.sync.dma_start(out=svec, in_=selv.rearrange("(t p) -> p t", p=128))
            ones1 = cpool.tile([1, 128], F32R, name="ones1")
            nc.sync.dma_start(out=ones1, in_=onesrow[None, :])

            wsb = []
            for h in range(NHC):
                w = qkpool.tile([128, H], F16, name=f"wsb{h}")
                nc.sync.dma_start(out=w, in_=woh[h * HD:(h + 1) * HD, :])
                wsb.append(w)

            svec32 = cpool.tile([128, KC], F32, name="svec32")
            nc.vector.tensor_copy(svec32, svec)
            ones = cpool.tile([128, 1], F16, name="ones")
            nc.vector.memset(ones, 1.0)
            ident = cpool.tile([128, 128], F16, name="ident")
            make_identity(nc, ident)
            iota = cpool.tile([128, S], F16, name="iota")
            nc.gpsimd.iota(
                iota, pattern=[[1, S]], base=0, channel_multiplier=0,
                allow_small_or_imprecise_dtypes=True,
            )

            aon = [aopool.tile([128, S], F16, name=f"aon{h}") for h in range(NHC)]
            vhf = [vhpool.tile([128, S], F16, name=f"vhf{h}") for h in range(NHC)]
            vsl = [vhpool.tile([128, S], F16, name=f"vsl{h}") for h in range(NHC)]

            def normalize(h, qc, avp, den):
                q0 = qc * 512
                dq = drpool.tile([1, 512], F32, tag="dq", name="dq")
                nc.scalar.copy(dq, den[0:1, :])
                rq = drpool.tile([1, 512], F32, tag="rq", name="rq")
                rs = drpool.tile([1, 512], F32, tag="rs", name="rs")
                nc.vector.reciprocal_approx_accurate(rq, dq, rs)
                rcq = drpool.tile([1, 512], F32R, tag="rcq", name="rcq")
                nc.vector.tensor_copy(rcq, rq)
                rb = pspool.tile([128, 512], F32, tag="sc", bufs=3, name="rb")
                nc.tensor.matmul(rb, ones1, rcq, start=True, stop=True)
                rbs = drpool.tile([128, 512], F32, tag="rbs", name="rbs")
                nc.scalar.copy(rbs, rb)
                nc.vector.scalar_tensor_tensor(
                    aon[h][:, q0:q0 + 512], rbs, 1.0, avp,
                    op0=OP.mult, op1=OP.mult,
                )

            def outproj(qc):
                """Output projection for the 4 query tiles of chunk qc,
                accumulating both heads; fp16 stage -> one DMA per tile."""
                for qt in range(qc * NQ, qc * NQ + NQ):
                    ostage = ostpool.tile([128, H], F16, tag="ost", name="ost")
                    for oc in range(OCC):
                        wop = pspool.tile([128, 512], F32, tag="wo", bufs=2,
                                          name="wo")
                        for h in range(NHC):
                            nc.tensor.matmul(
                                wop, aon[h][:, qt * 128:(qt + 1) * 128],
                                wsb[h][:, oc * 512:(oc + 1) * 512],
                                start=(h == 0), stop=(h == NHC - 1),
                            )
                        if oc % 2 == 0:
                            nc.vector.tensor_copy(
                                ostage[:, oc * 512:(oc + 1) * 512], wop
                            )
                        else:
                            nc.scalar.copy(
                                ostage[:, oc * 512:(oc + 1) * 512], wop
                            )
                    nc.sync.dma_start(
                        out=part[qt * 128:(qt + 1) * 128, :], in_=ostage
                    )

            for h in range(NHC):
                if h == 0:
                    vts = vts0
                else:
                    vts = vtpool.tile([128, S], F16, tag="vts", name="vts")
                    nc.sync.dma_start(out=vts, in_=vTh[h * HD:(h + 1) * HD, :])

                def transpose_batch(kcs):
                    # v tiles -> (k, hd) layout; vsl = v * selected. The tp
                    # psum shares the "wo" tag (outproj is temporally
                    # disjoint); copies go to gpsimd, which is mostly idle.
                    for kc in kcs:
                        tp = pspool.tile([128, 128], F16, tag="wo", bufs=2,
                                         name="tp")
                        nc.tensor.transpose(
                            tp, vts[:, kc * 128:(kc + 1) * 128], ident
                        )
                        if kc % 2 == 0:
                            nc.vector.tensor_copy(
                                vhf[h][:, kc * 128:(kc + 1) * 128], tp
                            )
                        else:
                            nc.scalar.copy(
                                vhf[h][:, kc * 128:(kc + 1) * 128], tp
                            )
                        nc.vector.tensor_scalar_mul(
                            vsl[h][:, kc * 128:(kc + 1) * 128],
                            vhf[h][:, kc * 128:(kc + 1) * 128],
                            svec32[:, kc:kc + 1],
                        )

                for qc in range(NQ):
                    transpose_batch(range(qc * NQ, qc * NQ + NQ))
                    q0 = qc * 512
                    kcm = (q0 + 511) // 128  # last causal k-tile
                    avp = pspool.tile([128, 512], F32, tag="av", bufs=2,
                                      name="avp")
                    den = pspool.tile([128, 512], F32, tag="den", bufs=1,
                                      name="den")
                    for kc in range(kcm + 1):
                        k0 = kc * 128
                        far = q0 > k0 + 127 + window
                        sps = pspool.tile([128, 512], F32, tag="sc", bufs=3,
                                          name="sps")
                        nc.tensor.matmul(
                            sps, ksb[h][:, k0:k0 + 128], qsb[h][:, q0:q0 + 512],
                            start=True, stop=True,
                        )
                        et = etpool.tile([128, 512], F16, tag="et", name="et")
                        nc.scalar.activation(et, sps, AF.Exp, scale=scale)
                        if q0 < k0 + 128:
                            nc.gpsimd.affine_select(
                                out=et, in_=et, compare_op=OP.is_ge, fill=0.0,
                                base=q0 - k0, channel_multiplier=-1,
                                pattern=[[1, 512]],
                            )
                        elif not far and q0 + 511 > k0 + window:
                            nc.vector.scalar_tensor_tensor(
                                et, iota[:, q0:q0 + 512], hvec[:, kc:kc + 1], et,
                                op0=OP.is_le, op1=OP.mult,
                            )
                        nc.tensor.matmul(
                            avp, (vsl if far else vhf)[h][:, k0:k0 + 128], et,
                            start=(kc == 0), stop=(kc == kcm),
                        )
                        nc.tensor.matmul(
                            den[0:1, :],
                            svec[:, kc:kc + 1] if far else ones, et,
                            start=(kc == 0), stop=(kc == kcm),
                            tile_position=(0, 0),
                        )
                    normalize(h, qc, avp, den)
                    if h == NHC - 1:
                        outproj(qc)
    nc.compile()
    return nc


_CACHE = {}


def _get(name, builder, *args):
    key = (name,) + args
    if key not in _CACHE:
        _CACHE[key] = builder(*args)
    return _CACHE[key]


def _run(nc, in_maps):
    res = run_bass_kernel_spmd(
        nc, in_maps, core_ids=list(range(N_CORES)), trace=_TRACE["on"]
    )
    if _TRACE["on"] and res.exec_time_ns is not None:
        _TRACE["exec_ns"].append(res.exec_time_ns)
    return res.results


def kernel(hidden_states, Wq, Wk, Wv, Wo, Wq_ind, Wk_ind, head_weights,
           temperature_param):
    hidden_states = np.asarray(hidden_states, dtype=FP32)
    Wq, Wk, Wv, Wo = (np.asarray(a, dtype=FP32) for a in (Wq, Wk, Wv, Wo))
    Wq_ind = np.asarray(Wq_ind, dtype=FP32)
    Wk_ind = np.asarray(Wk_ind, dtype=FP32)
    head_weights = np.asarray(head_weights, dtype=FP32)

    B, S, H = hidden_states.shape
    assert B == 1 and H == HIDDEN
    CS = H // N_CORES
    hidT = np.ascontiguousarray(hidden_states[0].T)
    # host-fused indexer weights (fp64 for exactness)
    Wfq = (Wq.astype(np.float64) @ Wq_ind.astype(np.float64)).astype(FP32)
    Wfk = (Wk.astype(np.float64) @ Wk_ind.astype(np.float64)).astype(FP32)

    # ---- launch A ----
    nca = _get("a", build_a, S, H, CS)
    ina = [
        {
            "hidT": hidT,
            "wq": np.ascontiguousarray(Wq[:, c * CS:(c + 1) * CS]),
            "wk": np.ascontiguousarray(Wk[:, c * CS:(c + 1) * CS]),
            "wv": np.ascontiguousarray(Wv[:, c * CS:(c + 1) * CS]),
            "wfq": np.ascontiguousarray(Wfq[:, c * CS:(c + 1) * CS]),
            "wfk": np.ascontiguousarray(Wfk[:, c * CS:(c + 1) * CS]),
        }
        for c in range(N_CORES)
    ]
    ra = _run(nca, ina)

    rel = np.zeros(S, dtype=np.float64)
    for c in range(N_CORES):
        rel += float(head_weights[c]) * ra[c]["rel"].astype(np.float64)
    # exp(-temp) scaling is monotone; irrelevant for top-k selection.

    k_sel = min(MAX_SELECTED, S)
    top_idx = np.argpartition(-rel, k_sel - 1)[:k_sel]
    selected = np.zeros(S, dtype=bool)
    selected[top_idx] = True

    # ---- launch B ----
    BIG = float(2 * S + 1024)
    hi = np.where(selected, BIG, np.arange(S, dtype=np.float64) + LOCAL_WINDOW)
    inb = [
        {
            "qTh": ra[c]["qT"],
            "kTh": ra[c]["kT"],
            "vTh": ra[c]["vT"],
            "woh": Wo[c * CS:(c + 1) * CS].astype(np.float16),
            "hivec": hi.astype(np.float16),
            "selv": selected.astype(np.float16),
            "onesrow": np.ones(128, dtype=np.float32),
        }
        for c in range(N_CORES)
    ]
    ncb = _get("b", build_b, S, H, NUM_HEADS // N_CORES, HEAD_DIM, LOCAL_WINDOW)
    rb = _run(ncb, inb)
    out = np.zeros((S, H), dtype=np.float32)
    for c in range(N_CORES):
        out += rb[c]["part"].astype(np.float32)
    return out.reshape(B, S, H)


# revision 18
# speedup vs baseline: 1.4116x; 1.0550x over previous
"""DeepSeek sparse attention on 8 Trainium2 NeuronCores (Bass/Tile).

Two SPMD launches:

  A (projections + lightning indexer, column/head-parallel): core c computes
     the 256-column slice (= its 2 attention heads) of q/k/v as fp16 (256,S)
     from f32r hidden^T resident in SBUF, PLUS the indexer-head-c projections
     qp_c/kp_c (256,S) using HOST-FUSED weights Wq@Wq_ind / Wk@Wk_ind (f32r,
     full precision — the top-k selection needs ~1e-5 relative accuracy), and
     rel_c[t] = sum_k relu(qp_c[t] . kp_c[k]) via PE + one relu-accumulate
     ACT pass per 128-token tile. Indexer passes run first so they overlap
     the hidden DMA window; scores interleave with the q/k/v passes.
  host: rel = sum_c w_c rel_c; top-1024 -> selected mask; hi/sel vectors.
  B (attention, head-parallel): core c feeds ITS OWN fp16 q/k/v slices from
     launch A straight back (no concat), computes causal/local/selected
     masked softmax attention for heads 2c,2c+1 and the partial output
     projection (S,H) in fp16; host sums the 8 fp16 partials in fp32.

All matmuls f32r or fp16 (1 PE cycle/row at N=512). fp16 everywhere in B
(calibrated: bf16 q/k/v + 16-bit partial store => 2.9e-3 rel err; fp16 is
strictly tighter; indexer stays f32r — one top-k swap costs ~1.5e-2).
"""

import math

import numpy as np

import concourse.bass as bass
import concourse.mybir as mybir
from concourse import bacc
from concourse.tile import TileContext
from concourse.masks import make_identity
from concourse.bass_utils import run_bass_kernel_spmd

# Problem constants (hardcoded per contract)
HIDDEN = 2048
NUM_HEADS = 16
HEAD_DIM = 128
NUM_IND_HEADS = 8
IND_DIM = HIDDEN // NUM_IND_HEADS  # 256
MAX_SELECTED = 1024
LOCAL_WINDOW = 512
N_CORES = 8

F32 = mybir.dt.float32
F32R = mybir.dt.float32r
F16 = mybir.dt.float16
FP32 = np.float32

_TRACE = {"on": False, "exec_ns": []}


def build_a(S=2048, H=HIDDEN, CS=HIDDEN // N_CORES):
    """Per-core: q/k/v column slices (CS,S) fp16 + indexer rel_c (S) f32."""
    nc = bacc.Bacc("TRN2", target_bir_lowering=False, debug=False)
    HT, MC, NQ, QT = H // 128, CS // 128, S // 512, S // 128
    hidT = nc.dram_tensor("hidT", [H, S], F32R, kind="ExternalInput")
    wq = nc.dram_tensor("wq", [H, CS], F32R, kind="ExternalInput")
    wk = nc.dram_tensor("wk", [H, CS], F32R, kind="ExternalInput")
    wv = nc.dram_tensor("wv", [H, CS], F32R, kind="ExternalInput")
    wfq = nc.dram_tensor("wfq", [H, CS], F32R, kind="ExternalInput")
    wfk = nc.dram_tensor("wfk", [H, CS], F32R, kind="ExternalInput")
    qT = nc.dram_tensor("qT", [CS, S], F16, kind="ExternalOutput")
    kT = nc.dram_tensor("kT", [CS, S], F16, kind="ExternalOutput")
    vT = nc.dram_tensor("vT", [CS, S], F16, kind="ExternalOutput")
    rel = nc.dram_tensor("rel", [S], F32, kind="ExternalOutput")

    G = 8          # hidden chunks
    TG = HT // G   # strips per chunk

    with TileContext(nc) as tc:
        with (
            tc.tile_pool(name="hid", bufs=1) as hpool,
            tc.tile_pool(name="wt", bufs=2) as wpool,
            tc.tile_pool(name="proj", bufs=1) as ppool,
            tc.tile_pool(name="st", bufs=2) as stpool,
            tc.tile_pool(name="scr", bufs=1) as scrpool,
            tc.tile_pool(name="rm", bufs=1) as rmpool,
            tc.tile_pool(name="ps", bufs=1, space="PSUM") as pspool,
        ):
            # ---- input DMAs. Order matters: the first matmul needs the
            # first half of wfq plus hidden chunk 0, so those go first; wfk
            # comes after the hidden chunks (first needed ~50us in); wq/wk/wv
            # are issued later, at the program points where their weight-pool
            # slot is freed (avoids WAR stalls on the slot).
            def load_w(wdram, dt=F32R, halves=1):
                wr = wpool.tile([128, HT * CS], dt, tag="w", name="w")
                hh = HT // halves
                for i in range(halves):
                    nc.sync.dma_start(
                        out=wr[:, i * hh * CS:(i + 1) * hh * CS].rearrange(
                            "p (t c) -> p t c", t=hh
                        ),
                        in_=wdram[i * hh * 128:(i + 1) * hh * 128, :].rearrange(
                            "(t p) c -> p t c", p=128
                        ),
                    )
                return wr

            hidc = [hpool.tile([128, TG * S], F32R, name=f"hidc{g}")
                    for g in range(G)]

            def load_hid(g):
                nc.sync.dma_start(
                    out=hidc[g].rearrange("p (t s) -> p t s", t=TG),
                    in_=hidT[g * TG * 128:(g + 1) * TG * 128, :].rearrange(
                        "(t p) s -> p t s", p=128
                    ),
                )

            wfq_t = wpool.tile([128, HT * CS], F32R, tag="w", name="w")
            HH = HT // 2
            nc.sync.dma_start(
                out=wfq_t[:, :HH * CS].rearrange("p (t c) -> p t c", t=HH),
                in_=wfq[:HH * 128, :].rearrange("(t p) c -> p t c", p=128),
            )
            load_hid(0)
            nc.sync.dma_start(
                out=wfq_t[:, HH * CS:].rearrange("p (t c) -> p t c", t=HH),
                in_=wfq[HH * 128:, :].rearrange("(t p) c -> p t c", p=128),
            )
            for g in range(1, G):
                load_hid(g)
            wfk_t = load_w(wfk)

            # resident f32r indexer projections qp^T/kp^T (2 x 128 x S each)
            qpt = [ppool.tile([128, S], F32R, name=f"qpt{m}") for m in range(MC)]
            kpt = [ppool.tile([128, S], F32R, name=f"kpt{m}") for m in range(MC)]

            # psum regions: 4 banks for projection passes ("pj"), and one
            # [128, S] region ("scr") that serves double duty: the kp passes
            # accumulate in its 512-slices, and the indexer-score tiles use
            # it whole.
            def pj_psums():
                return [
                    pspool.tile([128, 512], F32, tag=f"pj{i}", name=f"pj{i}")
                    for i in range(NQ)
                ]

            def scr_psum():
                return pspool.tile([128, S], F32, tag="scr", name="scr")

            relmat = rmpool.tile([128, QT], F32, name="relmat")
            scratch = scrpool.tile([128, S], F16, name="scratch")

            score_state = {"next": 0}

            def emit_score_qt():
                """Indexer scores for one 128-token tile: 8 matmuls into the
                scr psum region + one relu-accumulate -> relmat column."""
                qt = score_state["next"]
                if qt >= QT:
                    return False
                score_state["next"] += 1
                sps = scr_psum()
                for d in range(MC):
                    for kc in range(NQ):
                        nc.tensor.matmul(
                            sps[:, kc * 512:(kc + 1) * 512],
                            qpt[d][:, qt * 128:(qt + 1) * 128],
                            kpt[d][:, kc * 512:(kc + 1) * 512],
                            start=(d == 0), stop=(d == MC - 1),
                        )
                nc.scalar.activation(
                    scratch, sps, mybir.ActivationFunctionType.Relu,
                    accum_out=relmat[:, qt:qt + 1],
                )
                return True

            def sl(psums, qc):
                return (psums[qc] if isinstance(psums, list)
                        else psums[:, qc * 512:(qc + 1) * 512])

            def proj_passes(groups, score_slots=()):
                """Interleaved m-tile passes: each group = (wtile, mc, psums,
                finish). Strips advance together so every group progresses
                chunk-by-chunk behind the hidden DMA."""
                for t in range(HT):
                    for wtile, mc, psums, _ in groups:
                        lhsT = wtile[:, t * CS + mc * 128:
                                     t * CS + mc * 128 + 128]
                        rhs = hidc[t // TG]
                        tl = t % TG
                        for qc in range(NQ):
                            nc.tensor.matmul(
                                sl(psums, qc), lhsT,
                                rhs[:, tl * S + qc * 512:
                                    tl * S + qc * 512 + 512],
                                start=(t == 0), stop=(t == HT - 1),
                            )
                    if t in score_slots:
                        emit_score_qt()
                for _, _, _, finish in groups:
                    finish()

            def copy_to(dst, psums):
                def fin():
                    for qc in range(NQ):
                        eng = nc.vector if qc % 2 == 0 else nc.scalar
                        if eng is nc.vector:
                            nc.vector.tensor_copy(
                                dst[:, qc * 512:(qc + 1) * 512], sl(psums, qc)
                            )
                        else:
                            nc.scalar.copy(
                                dst[:, qc * 512:(qc + 1) * 512], sl(psums, qc)
                            )
                return fin

            # ---- indexer projection passes; the qp-m0/m1 pair overlaps the
            # hidden-DMA window (both read wfq, which arrives first).
            pj = pj_psums()
            sc = scr_psum()
            proj_passes([
                (wfq_t, 0, pj, copy_to(qpt[0], pj)),
                (wfq_t, 1, sc, copy_to(qpt[1], sc)),
            ])
            # wfq slot free now -> issue wq load
            wq_t = load_w(wq)
            pj = pj_psums()
            sc = scr_psum()
            proj_passes([
                (wfk_t, 0, pj, copy_to(kpt[0], pj)),
                (wfk_t, 1, sc, copy_to(kpt[1], sc)),
            ])
            wk_t = load_w(wk)

            # ---- q/k/v passes (fp16 weights) with indexer scores interleaved
            def store_pass(wtile, mc, odram):
                psums = pj_psums()
                stage = stpool.tile([128, S], F16, tag="st", name="st")

                def fin():
                    for qc in range(NQ):
                        if qc % 2 == 0:
                            nc.vector.tensor_copy(
                                stage[:, qc * 512:(qc + 1) * 512], psums[qc]
                            )
                        else:
                            nc.scalar.copy(
                                stage[:, qc * 512:(qc + 1) * 512], psums[qc]
                            )
                    nc.sync.dma_start(
                        out=odram[mc * 128:(mc + 1) * 128, :], in_=stage
                    )
                proj_passes([(wtile, mc, psums, fin)],
                            score_slots=(1, 6, 11))

            store_pass(wq_t, 0, qT)
            store_pass(wq_t, 1, qT)
            wv_t = load_w(wv)
            store_pass(wk_t, 0, kT)
            store_pass(wk_t, 1, kT)
            store_pass(wv_t, 0, vT)
            store_pass(wv_t, 1, vT)
            while emit_score_qt():
                pass

            nc.sync.dma_start(
                out=rel.rearrange("(t p) -> p t", p=128), in_=relmat
            )
    nc.compile()
    return nc


def build_b(S=2048, H=HIDDEN, NHC=NUM_HEADS // N_CORES, HD=HEAD_DIM,
            window=LOCAL_WINDOW):
    """Per-core attention for 2 heads + fp16 partial output projection."""
    nc = bacc.Bacc("TRN2", target_bir_lowering=False, debug=False)
    KC, NQ, QT, OCC = S // 128, S // 512, S // 128, H // 512
    qTh = nc.dram_tensor("qTh", [NHC * HD, S], F16, kind="ExternalInput")
    kTh = nc.dram_tensor("kTh", [NHC * HD, S], F16, kind="ExternalInput")
    vTh = nc.dram_tensor("vTh", [NHC * HD, S], F16, kind="ExternalInput")
    woh = nc.dram_tensor("woh", [NHC * HD, H], F16, kind="ExternalInput")
    hivec = nc.dram_tensor("hivec", [S], F16, kind="ExternalInput")
    selv = nc.dram_tensor("selv", [S], F16, kind="ExternalInput")
    part = nc.dram_tensor("part", [S, H], F16, kind="ExternalOutput")

    scale = 1.0 / math.sqrt(HD)
    AF = mybir.ActivationFunctionType
    OP = mybir.AluOpType

    with TileContext(nc) as tc:
        with (
            tc.tile_pool(name="const", bufs=1) as cpool,
            tc.tile_pool(name="qk", bufs=1) as qkpool,
            tc.tile_pool(name="vt", bufs=2) as vtpool,
            tc.tile_pool(name="vh", bufs=1) as vhpool,
            tc.tile_pool(name="et", bufs=3) as etpool,
            tc.tile_pool(name="aon", bufs=1) as aopool,
            tc.tile_pool(name="dr", bufs=2) as drpool,
            tc.tile_pool(name="ost", bufs=2) as ostpool,
            tc.tile_pool(name="ps", bufs=1, space="PSUM") as pspool,
        ):
            # v first (transposes are at the head of the PE queue), then q/k
            # of head 0 so scoring starts right behind the transposes.
            vts0 = vtpool.tile([128, S], F16, tag="vts", name="vts")
            nc.sync.dma_start(out=vts0, in_=vTh[0:HD, :])
            qsb, ksb = [], []
            for h in range(NHC):
                k = qkpool.tile([128, S], F16, name=f"ksb{h}")
                nc.sync.dma_start(out=k, in_=kTh[h * HD:(h + 1) * HD, :])
                ksb.append(k)
                q = qkpool.tile([128, S], F16, name=f"qsb{h}")
                nc.sync.dma_start(out=q, in_=qTh[h * HD:(h + 1) * HD, :])
                qsb.append(q)

            hvec = cpool.tile([128, KC], F16, name="hvec")
            nc.sync.dma_start(out=hvec, in_=hivec.rearrange("(t p) -> p t", p=128))
            svec = cpool.tile([128, KC], F16, name="svec")
            nc.sync.dma_start(out=svec, in_=selv.rearrange("(t p) -> p t", p=128))

            wsb = []
            for h in range(NHC):
                w = qkpool.tile([128, H], F16, name=f"wsb{h}")
                nc.sync.dma_start(out=w, in_=woh[h * HD:(h + 1) * HD, :])
                wsb.append(w)

            svec32 = cpool.tile([128, KC], F32, name="svec32")
            nc.vector.tensor_copy(svec32, svec)
            ones = cpool.tile([128, 1], F16, name="ones")
            nc.vector.memset(ones, 1.0)
            ident = cpool.tile([128, 128], F16, name="ident")
            make_identity(nc, ident)
            iota = cpool.tile([128, S], F16, name="iota")
            nc.gpsimd.iota(
                iota, pattern=[[1, S]], base=0, channel_multiplier=0,
                allow_small_or_imprecise_dtypes=True,
            )

            aon = [aopool.tile([128, S], F16, name=f"aon{h}") for h in range(NHC)]
            vhf = [vhpool.tile([128, S], F16, name=f"vhf{h}") for h in range(NHC)]
            vsl = [vhpool.tile([128, S], F16, name=f"vsl{h}") for h in range(NHC)]

            def normalize(h, qc, avp, den):
                q0 = qc * 512
                dq = drpool.tile([1, 512], F32, tag="dq", name="dq")
                nc.scalar.copy(dq, den[0:1, :])
                rq = drpool.tile([1, 512], F32, tag="rq", name="rq")
                rs = drpool.tile([1, 512], F32, tag="rs", name="rs")
                nc.vector.reciprocal_approx_accurate(rq, dq, rs)
                rbs = drpool.tile([128, 512], F32, tag="rbs", name="rbs")
                nc.gpsimd.partition_broadcast(rbs, rq)
                nc.vector.scalar_tensor_tensor(
                    aon[h][:, q0:q0 + 512], rbs, 1.0, avp,
                    op0=OP.mult, op1=OP.mult,
                )

            def outproj(qc):
                """Output projection for the 4 query tiles of chunk qc,
                accumulating both heads; fp16 stage -> one DMA per tile."""
                for qt in range(qc * NQ, qc * NQ + NQ):
                    ostage = ostpool.tile([128, H], F16, tag="ost", name="ost")
                    for oc in range(OCC):
                        wop = pspool.tile([128, 512], F32, tag="wo", bufs=2,
                                          name="wo")
                        for h in range(NHC):
                            nc.tensor.matmul(
                                wop, aon[h][:, qt * 128:(qt + 1) * 128],
                                wsb[h][:, oc * 512:(oc + 1) * 512],
                                start=(h == 0), stop=(h == NHC - 1),
                            )
                        if oc % 2 == 0:
                            nc.vector.tensor_copy(
                                ostage[:, oc * 512:(oc + 1) * 512], wop
                            )
                        else:
                            nc.scalar.copy(
                                ostage[:, oc * 512:(oc + 1) * 512], wop
                            )
                    nc.sync.dma_start(
                        out=part[qt * 128:(qt + 1) * 128, :], in_=ostage
                    )

            for h in range(NHC):
                if h == 0:
                    vts = vts0
                else:
                    vts = vtpool.tile([128, S], F16, tag="vts", name="vts")
                    nc.sync.dma_start(out=vts, in_=vTh[h * HD:(h + 1) * HD, :])

                def transpose_batch(kcs):
                    # v tiles -> (k, hd) layout; vsl = v * selected. The tp
                    # psum shares the "wo" tag (outproj is temporally
                    # disjoint); copies go to gpsimd, which is mostly idle.
                    for kc in kcs:
                        tp = pspool.tile([128, 128], F16, tag="wo", bufs=2,
                                         name="tp")
                        nc.tensor.transpose(
                            tp, vts[:, kc * 128:(kc + 1) * 128], ident
                        )
                        if kc % 2 == 0:
                            nc.vector.tensor_copy(
                                vhf[h][:, kc * 128:(kc + 1) * 128], tp
                            )
                        else:
                            nc.scalar.copy(
                                vhf[h][:, kc * 128:(kc + 1) * 128], tp
                            )
                        nc.vector.tensor_scalar_mul(
                            vsl[h][:, kc * 128:(kc + 1) * 128],
                            vhf[h][:, kc * 128:(kc + 1) * 128],
                            svec32[:, kc:kc + 1],
                        )

                for qc in range(NQ):
                    transpose_batch(range(qc * NQ, qc * NQ + NQ))
                    q0 = qc * 512
                    kcm = (q0 + 511) // 128  # last causal k-tile
                    avp = pspool.tile([128, 512], F32, tag="av", bufs=2,
                                      name="avp")
                    den = pspool.tile([128, 512], F32, tag="den", bufs=1,
                                      name="den")
                    for kc in range(kcm + 1):
                        k0 = kc * 128
                        far = q0 > k0 + 127 + window
                        sps = pspool.tile([128, 512], F32, tag="sc", bufs=3,
                                          name="sps")
                        nc.tensor.matmul(
                            sps, ksb[h][:, k0:k0 + 128], qsb[h][:, q0:q0 + 512],
                            start=True, stop=True,
                        )
                        et = etpool.tile([128, 512], F16, tag="et", name="et")
                        nc.scalar.activation(et, sps, AF.Exp, scale=scale)
                        if q0 < k0 + 128:
                            nc.gpsimd.affine_select(
                                out=et, in_=et, compare_op=OP.is_ge, fill=0.0,
                                base=q0 - k0, channel_multiplier=-1,
                                pattern=[[1, 512]],
                            )
                        elif not far and q0 + 511 > k0 + window:
                            nc.vector.scalar_tensor_tensor(
                                et, iota[:, q0:q0 + 512], hvec[:, kc:kc + 1], et,
                                op0=OP.is_le, op1=OP.mult,
                            )
                        nc.tensor.matmul(
                            avp, (vsl if far else vhf)[h][:, k0:k0 + 128], et,
                            start=(kc == 0), stop=(kc == kcm),
                        )
                        nc.tensor.matmul(
                            den[0:1, :],
                            svec[:, kc:kc + 1] if far else ones, et,
                            start=(kc == 0), stop=(kc == kcm),
                            tile_position=(0, 0),
                        )
                    normalize(h, qc, avp, den)
                    if h == NHC - 1:
                        outproj(qc)
    nc.compile()
    return nc


_CACHE = {}


def _get(name, builder, *args):
    key = (name,) + args
    if key not in _CACHE:
        _CACHE[key] = builder(*args)
    return _CACHE[key]


def _run(nc, in_maps):
    res = run_bass_kernel_spmd(
        nc, in_maps, core_ids=list(range(N_CORES)), trace=_TRACE["on"]
    )
    if _TRACE["on"] and res.exec_time_ns is not None:
        _TRACE["exec_ns"].append(res.exec_time_ns)
    return res.results


def kernel(hidden_states, Wq, Wk, Wv, Wo, Wq_ind, Wk_ind, head_weights,
           temperature_param):
    hidden_states = np.asarray(hidden_states, dtype=FP32)
    Wq, Wk, Wv, Wo = (np.asarray(a, dtype=FP32) for a in (Wq, Wk, Wv, Wo))
    Wq_ind = np.asarray(Wq_ind, dtype=FP32)
    Wk_ind = np.asarray(Wk_ind, dtype=FP32)
    head_weights = np.asarray(head_weights, dtype=FP32)

    B, S, H = hidden_states.shape
    assert B == 1 and H == HIDDEN
    CS = H // N_CORES
    hidT = np.ascontiguousarray(hidden_states[0].T)
    # host-fused indexer weights (fp64 for exactness)
    Wfq = (Wq.astype(np.float64) @ Wq_ind.astype(np.float64)).astype(FP32)
    Wfk = (Wk.astype(np.float64) @ Wk_ind.astype(np.float64)).astype(FP32)

    # ---- launch A ----
    nca = _get("a", build_a, S, H, CS)
    ina = [
        {
            "hidT": hidT,
            "wq": np.ascontiguousarray(Wq[:, c * CS:(c + 1) * CS]),
            "wk": np.ascontiguousarray(Wk[:, c * CS:(c + 1) * CS]),
            "wv": np.ascontiguousarray(Wv[:, c * CS:(c + 1) * CS]),
            "wfq": np.ascontiguousarray(Wfq[:, c * CS:(c + 1) * CS]),
            "wfk": np.ascontiguousarray(Wfk[:, c * CS:(c + 1) * CS]),
        }
        for c in range(N_CORES)
    ]
    ra = _run(nca, ina)

    rel = np.zeros(S, dtype=np.float64)
    for c in range(N_CORES):
        rel += float(head_weights[c]) * ra[c]["rel"].astype(np.float64)
    # exp(-temp) scaling is monotone; irrelevant for top-k selection.

    k_sel = min(MAX_SELECTED, S)
    top_idx = np.argpartition(-rel, k_sel - 1)[:k_sel]
    selected = np.zeros(S, dtype=bool)
    selected[top_idx] = True

    # ---- launch B ----
    BIG = float(2 * S + 1024)
    hi = np.where(selected, BIG, np.arange(S, dtype=np.float64) + LOCAL_WINDOW)
    inb = [
        {
            "qTh": ra[c]["qT"],
            "kTh": ra[c]["kT"],
            "vTh": ra[c]["vT"],
            "woh": Wo[c * CS:(c + 1) * CS].astype(np.float16),
            "hivec": hi.astype(np.float16),
            "selv": selected.astype(np.float16),
        }
        for c in range(N_CORES)
    ]
    ncb = _get("b", build_b, S, H, NUM_HEADS // N_CORES, HEAD_DIM, LOCAL_WINDOW)
    rb = _run(ncb, inb)
    out = np.zeros((S, H), dtype=np.float32)
    for c in range(N_CORES):
        out += rb[c]["part"].astype(np.float32)
    return out.reshape(B, S, H)


# revision 24
# speedup vs baseline: 1.4380x; 1.0187x over previous
"""DeepSeek sparse attention on 8 Trainium2 NeuronCores (Bass/Tile).

Two SPMD launches:

  A (projections + lightning indexer, column/head-parallel): core c computes
     the 256-column slice (= its 2 attention heads) of q/k/v as fp16 (256,S)
     from f32r hidden^T resident in SBUF, PLUS the indexer-head-c projections
     qp_c/kp_c (256,S) using HOST-FUSED weights Wq@Wq_ind / Wk@Wk_ind (f32r,
     full precision — the top-k selection needs ~1e-5 relative accuracy), and
     rel_c[t] = sum_k relu(qp_c[t] . kp_c[k]) via PE + one relu-accumulate
     ACT pass per 128-token tile. Indexer passes run first so they overlap
     the hidden DMA window; scores interleave with the q/k/v passes.
  host: rel = sum_c w_c rel_c; top-1024 -> selected mask; hi/sel vectors.
  B (attention, head-parallel): core c feeds ITS OWN fp16 q/k/v slices from
     launch A straight back (no concat), computes causal/local/selected
     masked softmax attention for heads 2c,2c+1 and the partial output
     projection (S,H) in fp16; host sums the 8 fp16 partials in fp32.

All matmuls f32r or fp16 (1 PE cycle/row at N=512). fp16 everywhere in B
(calibrated: bf16 q/k/v + 16-bit partial store => 2.9e-3 rel err; fp16 is
strictly tighter; indexer stays f32r — one top-k swap costs ~1.5e-2).
"""

import math

import numpy as np

import concourse.bass as bass
import concourse.mybir as mybir
from concourse import bacc
from concourse.tile import TileContext
from concourse.masks import make_identity
from concourse.bass_utils import run_bass_kernel_spmd

# Problem constants (hardcoded per contract)
HIDDEN = 2048
NUM_HEADS = 16
HEAD_DIM = 128
NUM_IND_HEADS = 8
IND_DIM = HIDDEN // NUM_IND_HEADS  # 256
MAX_SELECTED = 1024
LOCAL_WINDOW = 512
N_CORES = 8

F32 = mybir.dt.float32
F32R = mybir.dt.float32r
F16 = mybir.dt.float16
FP32 = np.float32

_TRACE = {"on": False, "exec_ns": []}


def build_a(S=2048, H=HIDDEN, CS=HIDDEN // N_CORES):
    """Per-core: q/k/v column slices (CS,S) fp16 + indexer rel_c (S) f32."""
    nc = bacc.Bacc("TRN2", target_bir_lowering=False, debug=False)
    HT, MC, NQ, QT = H // 128, CS // 128, S // 512, S // 128
    hidT = nc.dram_tensor("hidT", [H, S], F32R, kind="ExternalInput")
    wq = nc.dram_tensor("wq", [H, CS], F32R, kind="ExternalInput")
    wk = nc.dram_tensor("wk", [H, CS], F32R, kind="ExternalInput")
    wv = nc.dram_tensor("wv", [H, CS], F32R, kind="ExternalInput")
    wfq = nc.dram_tensor("wfq", [H, CS], F32R, kind="ExternalInput")
    wfk = nc.dram_tensor("wfk", [H, CS], F32R, kind="ExternalInput")
    qT = nc.dram_tensor("qT", [CS, S], F16, kind="ExternalOutput")
    kT = nc.dram_tensor("kT", [CS, S], F16, kind="ExternalOutput")
    vT = nc.dram_tensor("vT", [CS, S], F16, kind="ExternalOutput")
    rel = nc.dram_tensor("rel", [S], F32, kind="ExternalOutput")

    G = 8          # hidden chunks
    TG = HT // G   # strips per chunk

    with TileContext(nc) as tc:
        with (
            tc.tile_pool(name="hid", bufs=1) as hpool,
            tc.tile_pool(name="wt", bufs=2) as wpool,
            tc.tile_pool(name="proj", bufs=1) as ppool,
            tc.tile_pool(name="st", bufs=2) as stpool,
            tc.tile_pool(name="scr", bufs=1) as scrpool,
            tc.tile_pool(name="rm", bufs=1) as rmpool,
            tc.tile_pool(name="ps", bufs=1, space="PSUM") as pspool,
        ):
            # ---- input DMAs. Order matters: the first matmul needs the
            # first half of wfq plus hidden chunk 0, so those go first; wfk
            # comes after the hidden chunks (first needed ~50us in); wq/wk/wv
            # are issued later, at the program points where their weight-pool
            # slot is freed (avoids WAR stalls on the slot).
            def load_w(wdram, dt=F32R, halves=1):
                wr = wpool.tile([128, HT * CS], dt, tag="w", name="w")
                hh = HT // halves
                for i in range(halves):
                    nc.sync.dma_start(
                        out=wr[:, i * hh * CS:(i + 1) * hh * CS].rearrange(
                            "p (t c) -> p t c", t=hh
                        ),
                        in_=wdram[i * hh * 128:(i + 1) * hh * 128, :].rearrange(
                            "(t p) c -> p t c", p=128
                        ),
                    )
                return wr

            # hidden chunks: strips [1,1,2,2,2,2,2,2,2] so the first
            # matmul only waits on one 1MB strip; wfq loads in quarters.
            CHUNKS = [1, 1] + [2] * 7
            CUM = [0]
            for n in CHUNKS:
                CUM.append(CUM[-1] + n)
            strip2chunk = {}
            for g, n in enumerate(CHUNKS):
                for t in range(CUM[g], CUM[g + 1]):
                    strip2chunk[t] = (g, t - CUM[g])
            hidc = [hpool.tile([128, CHUNKS[g] * S], F32R, name=f"hidc{g}")
                    for g in range(len(CHUNKS))]

            def load_hid(g):
                n = CHUNKS[g]
                nc.sync.dma_start(
                    out=hidc[g].rearrange("p (t s) -> p t s", t=n),
                    in_=hidT[CUM[g] * 128:CUM[g + 1] * 128, :].rearrange(
                        "(t p) s -> p t s", p=128
                    ),
                )

            wfq_t = wpool.tile([128, HT * CS], F32R, tag="w", name="w")
            QQ = HT // 4
            nc.sync.dma_start(
                out=wfq_t[:, :QQ * CS].rearrange("p (t c) -> p t c", t=QQ),
                in_=wfq[:QQ * 128, :].rearrange("(t p) c -> p t c", p=128),
            )
            load_hid(0)
            nc.sync.dma_start(
                out=wfq_t[:, QQ * CS:2 * QQ * CS].rearrange(
                    "p (t c) -> p t c", t=QQ),
                in_=wfq[QQ * 128:2 * QQ * 128, :].rearrange(
                    "(t p) c -> p t c", p=128),
            )
            load_hid(1)
            nc.sync.dma_start(
                out=wfq_t[:, 2 * QQ * CS:].rearrange(
                    "p (t c) -> p t c", t=2 * QQ),
                in_=wfq[2 * QQ * 128:, :].rearrange(
                    "(t p) c -> p t c", p=128),
            )
            for g in range(2, len(CHUNKS)):
                load_hid(g)
            wfk_t = load_w(wfk)

            # resident f32r indexer projections qp^T/kp^T (2 x 128 x S each)
            qpt = [ppool.tile([128, S], F32R, name=f"qpt{m}") for m in range(MC)]
            kpt = [ppool.tile([128, S], F32R, name=f"kpt{m}") for m in range(MC)]

            # psum regions: 4 banks for projection passes ("pj"), and one
            # [128, S] region ("scr") that serves double duty: the kp passes
            # accumulate in its 512-slices, and the indexer-score tiles use
            # it whole.
            def pj_psums():
                return [
                    pspool.tile([128, 512], F32, tag=f"pj{i}", name=f"pj{i}")
                    for i in range(NQ)
                ]

            def scr_psum():
                return pspool.tile([128, S], F32, tag="scr", name="scr")

            relmat = rmpool.tile([128, QT], F32, name="relmat")
            scratch = scrpool.tile([128, S], F16, name="scratch")

            score_state = {"next": 0}

            def emit_score_qt():
                """Indexer scores for one 128-token tile: 8 matmuls into the
                scr psum region + one relu-accumulate -> relmat column."""
                qt = score_state["next"]
                if qt >= QT:
                    return False
                score_state["next"] += 1
                sps = scr_psum()
                for d in range(MC):
                    for kc in range(NQ):
                        nc.tensor.matmul(
                            sps[:, kc * 512:(kc + 1) * 512],
                            qpt[d][:, qt * 128:(qt + 1) * 128],
                            kpt[d][:, kc * 512:(kc + 1) * 512],
                            start=(d == 0), stop=(d == MC - 1),
                        )
                nc.scalar.activation(
                    scratch, sps, mybir.ActivationFunctionType.Relu,
                    accum_out=relmat[:, qt:qt + 1],
                )
                return True

            def sl(psums, qc):
                return (psums[qc] if isinstance(psums, list)
                        else psums[:, qc * 512:(qc + 1) * 512])

            def proj_passes(groups, score_slots=()):
                """Interleaved m-tile passes: each group = (wtile, mc, psums,
                finish). Strips advance together so every group progresses
                chunk-by-chunk behind the hidden DMA."""
                for t in range(HT):
                    for wtile, mc, psums, _ in groups:
                        lhsT = wtile[:, t * CS + mc * 128:
                                     t * CS + mc * 128 + 128]
                        g, tl = strip2chunk[t]
                        rhs = hidc[g]
                        for qc in range(NQ):
                            nc.tensor.matmul(
                                sl(psums, qc), lhsT,
                                rhs[:, tl * S + qc * 512:
                                    tl * S + qc * 512 + 512],
                                start=(t == 0), stop=(t == HT - 1),
                            )
                    if t in score_slots:
                        emit_score_qt()
                for _, _, _, finish in groups:
                    finish()

            def copy_to(dst, psums):
                def fin():
                    for qc in range(NQ):
                        eng = nc.vector if qc % 2 == 0 else nc.scalar
                        if eng is nc.vector:
                            nc.vector.tensor_copy(
                                dst[:, qc * 512:(qc + 1) * 512], sl(psums, qc)
                            )
                        else:
                            nc.scalar.copy(
                                dst[:, qc * 512:(qc + 1) * 512], sl(psums, qc)
                            )
                return fin

            # ---- indexer projection passes; the qp-m0/m1 pair overlaps the
            # hidden-DMA window (both read wfq, which arrives first).
            pj = pj_psums()
            sc = scr_psum()
            proj_passes([
                (wfq_t, 0, pj, copy_to(qpt[0], pj)),
                (wfq_t, 1, sc, copy_to(qpt[1], sc)),
            ])
            # wfq slot free now -> issue wq load
            wq_t = load_w(wq)
            pj = pj_psums()
            sc = scr_psum()
            proj_passes([
                (wfk_t, 0, pj, copy_to(kpt[0], pj)),
                (wfk_t, 1, sc, copy_to(kpt[1], sc)),
            ])
            wk_t = load_w(wk)

            # ---- q/k/v passes (fp16 weights) with indexer scores interleaved
            def store_pass(wtile, mc, odram):
                psums = pj_psums()
                stage = stpool.tile([128, S], F16, tag="st", name="st")

                def fin():
                    for qc in range(NQ):
                        if qc % 2 == 0:
                            nc.vector.tensor_copy(
                                stage[:, qc * 512:(qc + 1) * 512], psums[qc]
                            )
                        else:
                            nc.scalar.copy(
                                stage[:, qc * 512:(qc + 1) * 512], psums[qc]
                            )
                    nc.sync.dma_start(
                        out=odram[mc * 128:(mc + 1) * 128, :], in_=stage
                    )
                proj_passes([(wtile, mc, psums, fin)],
                            score_slots=(1, 6, 11))

            store_pass(wq_t, 0, qT)
            store_pass(wq_t, 1, qT)
            wv_t = load_w(wv)
            store_pass(wk_t, 0, kT)
            store_pass(wk_t, 1, kT)
            store_pass(wv_t, 0, vT)
            store_pass(wv_t, 1, vT)
            while emit_score_qt():
                pass

            nc.sync.dma_start(
                out=rel.rearrange("(t p) -> p t", p=128), in_=relmat
            )
    nc.compile()
    return nc


def build_b(S=2048, H=HIDDEN, NHC=NUM_HEADS // N_CORES, HD=HEAD_DIM,
            window=LOCAL_WINDOW):
    """Per-core attention for 2 heads + fp16 partial output projection."""
    nc = bacc.Bacc("TRN2", target_bir_lowering=False, debug=False)
    KC, NQ, QT, OCC = S // 128, S // 512, S // 128, H // 512
    qTh = nc.dram_tensor("qTh", [NHC * HD, S], F16, kind="ExternalInput")
    kTh = nc.dram_tensor("kTh", [NHC * HD, S], F16, kind="ExternalInput")
    vTh = nc.dram_tensor("vTh", [NHC * HD, S], F16, kind="ExternalInput")
    woh = nc.dram_tensor("woh", [NHC * HD, H], F16, kind="ExternalInput")
    hivec = nc.dram_tensor("hivec", [S], F16, kind="ExternalInput")
    selv = nc.dram_tensor("selv", [S], F16, kind="ExternalInput")
    part = nc.dram_tensor("part", [S, H], F16, kind="ExternalOutput")

    scale = 1.0 / math.sqrt(HD)
    AF = mybir.ActivationFunctionType
    OP = mybir.AluOpType

    with TileContext(nc) as tc:
        with (
            tc.tile_pool(name="const", bufs=1) as cpool,
            tc.tile_pool(name="qk", bufs=1) as qkpool,
            tc.tile_pool(name="vt", bufs=2) as vtpool,
            tc.tile_pool(name="vh", bufs=1) as vhpool,
            tc.tile_pool(name="et", bufs=4) as etpool,
            tc.tile_pool(name="aon", bufs=1) as aopool,
            tc.tile_pool(name="dr", bufs=2) as drpool,
            tc.tile_pool(name="ost", bufs=2) as ostpool,
            tc.tile_pool(name="ps", bufs=1, space="PSUM") as pspool,
        ):
            # v first (transposes are at the head of the PE queue), then q/k
            # of head 0 so scoring starts right behind the transposes.
            vts0 = vtpool.tile([128, S], F16, tag="vts", name="vts")
            nc.sync.dma_start(out=vts0, in_=vTh[0:HD, :])
            qsb, ksb = [], []
            for h in range(NHC):
                k = qkpool.tile([128, S], F16, name=f"ksb{h}")
                nc.sync.dma_start(out=k, in_=kTh[h * HD:(h + 1) * HD, :])
                ksb.append(k)
                q = qkpool.tile([128, S], F16, name=f"qsb{h}")
                nc.sync.dma_start(out=q, in_=qTh[h * HD:(h + 1) * HD, :])
                qsb.append(q)

            hvec = cpool.tile([128, KC], F16, name="hvec")
            nc.sync.dma_start(out=hvec, in_=hivec.rearrange("(t p) -> p t", p=128))
            svec = cpool.tile([128, KC], F16, name="svec")
            nc.sync.dma_start(out=svec, in_=selv.rearrange("(t p) -> p t", p=128))

            wsb = []
            for h in range(NHC):
                w = qkpool.tile([128, H], F16, name=f"wsb{h}")
                nc.sync.dma_start(out=w, in_=woh[h * HD:(h + 1) * HD, :])
                wsb.append(w)

            svec32 = cpool.tile([128, KC], F32, name="svec32")
            nc.vector.tensor_copy(svec32, svec)
            ones = cpool.tile([128, 1], F16, name="ones")
            nc.vector.memset(ones, 1.0)
            ident = cpool.tile([128, 128], F16, name="ident")
            make_identity(nc, ident)
            iota = cpool.tile([128, S], F16, name="iota")
            nc.gpsimd.iota(
                iota, pattern=[[1, S]], base=0, channel_multiplier=0,
                allow_small_or_imprecise_dtypes=True,
            )

            aon = [aopool.tile([128, S], F16, name=f"aon{h}") for h in range(NHC)]
            vhf = [vhpool.tile([128, S], F16, name=f"vhf{h}") for h in range(NHC)]
            vsl = [vhpool.tile([128, S], F16, name=f"vsl{h}") for h in range(NHC)]

            def normalize(h, qc, avp, den):
                q0 = qc * 512
                dq = drpool.tile([1, 512], F32, tag="dq", name="dq")
                nc.scalar.copy(dq, den[0:1, :])
                rq = drpool.tile([1, 512], F32, tag="rq", name="rq")
                rs = drpool.tile([1, 512], F32, tag="rs", name="rs")
                nc.vector.reciprocal_approx_accurate(rq, dq, rs)
                rbs = drpool.tile([128, 512], F32, tag="rbs", name="rbs")
                nc.gpsimd.partition_broadcast(rbs, rq)
                nc.vector.scalar_tensor_tensor(
                    aon[h][:, q0:q0 + 512], rbs, 1.0, avp,
                    op0=OP.mult, op1=OP.mult,
                )

            def outproj(qc):
                """Output projection for the 4 query tiles of chunk qc,
                accumulating both heads; fp16 stage -> one DMA per tile."""
                for qt in range(qc * NQ, qc * NQ + NQ):
                    ostage = ostpool.tile([128, H], F16, tag="ost", name="ost")
                    for oc in range(OCC):
                        wop = pspool.tile([128, 512], F32, tag="wo", bufs=2,
                                          name="wo")
                        for h in range(NHC):
                            nc.tensor.matmul(
                                wop, aon[h][:, qt * 128:(qt + 1) * 128],
                                wsb[h][:, oc * 512:(oc + 1) * 512],
                                start=(h == 0), stop=(h == NHC - 1),
                            )
                        if oc % 2 == 0:
                            nc.vector.tensor_copy(
                                ostage[:, oc * 512:(oc + 1) * 512], wop
                            )
                        else:
                            nc.scalar.copy(
                                ostage[:, oc * 512:(oc + 1) * 512], wop
                            )
                    nc.sync.dma_start(
                        out=part[qt * 128:(qt + 1) * 128, :], in_=ostage
                    )

            for h in range(NHC):
                if h == 0:
                    vts = vts0
                else:
                    vts = vtpool.tile([128, S], F16, tag="vts", name="vts")
                    nc.sync.dma_start(out=vts, in_=vTh[h * HD:(h + 1) * HD, :])

                def transpose_batch(kcs):
                    # v tiles -> (k, hd) layout; vsl = v * selected. The tp
                    # psum shares the "wo" tag (outproj is temporally
                    # disjoint); copies go to gpsimd, which is mostly idle.
                    for kc in kcs:
                        tp = pspool.tile([128, 128], F16, tag="wo", bufs=2,
                                         name="tp")
                        nc.tensor.transpose(
                            tp, vts[:, kc * 128:(kc + 1) * 128], ident
                        )
                        if kc % 2 == 0:
                            nc.vector.tensor_copy(
                                vhf[h][:, kc * 128:(kc + 1) * 128], tp
                            )
                        else:
                            nc.scalar.copy(
                                vhf[h][:, kc * 128:(kc + 1) * 128], tp
                            )
                        nc.vector.tensor_scalar_mul(
                            vsl[h][:, kc * 128:(kc + 1) * 128],
                            vhf[h][:, kc * 128:(kc + 1) * 128],
                            svec32[:, kc:kc + 1],
                        )

                for qc in range(NQ):
                    q0 = qc * 512
                    kcm = (q0 + 511) // 128  # last causal k-tile
                    avp = pspool.tile([128, 512], F32, tag="av", bufs=2,
                                      name="avp")
                    den = pspool.tile([128, 512], F32, tag="den", bufs=1,
                                      name="den")

                    def score_tile(kc):
                        k0 = kc * 128
                        far = q0 > k0 + 127 + window
                        sps = pspool.tile([128, 512], F32, tag="sc", bufs=3,
                                          name="sps")
                        nc.tensor.matmul(
                            sps, ksb[h][:, k0:k0 + 128],
                            qsb[h][:, q0:q0 + 512],
                            start=True, stop=True,
                        )
                        et = etpool.tile([128, 512], F16, tag="et", name="et")
                        nc.scalar.activation(et, sps, AF.Exp, scale=scale)
                        if q0 < k0 + 128:
                            nc.gpsimd.affine_select(
                                out=et, in_=et, compare_op=OP.is_ge, fill=0.0,
                                base=q0 - k0, channel_multiplier=-1,
                                pattern=[[1, 512]],
                            )
                        elif not far and q0 + 511 > k0 + window:
                            nc.vector.scalar_tensor_tensor(
                                et, iota[:, q0:q0 + 512], hvec[:, kc:kc + 1],
                                et, op0=OP.is_le, op1=OP.mult,
                            )
                        return et, far

                    def av_den(kc, et, far):
                        k0 = kc * 128
                        nc.tensor.matmul(
                            avp, (vsl if far else vhf)[h][:, k0:k0 + 128], et,
                            start=(kc == 0), stop=(kc == kcm),
                        )
                        nc.tensor.matmul(
                            den[0:1, :],
                            svec[:, kc:kc + 1] if far else ones, et,
                            start=(kc == 0), stop=(kc == kcm),
                            tile_position=(0, 0),
                        )

                    if qc == 0:
                        # scores first: they only need q/k, so the PE can
                        # work while v arrives and the transposes run.
                        ets = [score_tile(kc) for kc in range(kcm + 1)]
                        transpose_batch(range(qc * NQ, qc * NQ + NQ))
                        for kc, (et, far) in enumerate(ets):
                            av_den(kc, et, far)
                    else:
                        transpose_batch(range(qc * NQ, qc * NQ + NQ))
                        for kc in range(kcm + 1):
                            et, far = score_tile(kc)
                            av_den(kc, et, far)
                    normalize(h, qc, avp, den)
                    if h == NHC - 1:
                        outproj(qc)
    nc.compile()
    return nc


_CACHE = {}


def _get(name, builder, *args):
    key = (name,) + args
    if key not in _CACHE:
        _CACHE[key] = builder(*args)
    return _CACHE[key]


def _run(nc, in_maps):
    res = run_bass_kernel_spmd(
        nc, in_maps, core_ids=list(range(N_CORES)), trace=_TRACE["on"]
    )
    if _TRACE["on"] and res.exec_time_ns is not None:
        _TRACE["exec_ns"].append(res.exec_time_ns)
    return res.results


def kernel(hidden_states, Wq, Wk, Wv, Wo, Wq_ind, Wk_ind, head_weights,
           temperature_param):
    hidden_states = np.asarray(hidden_states, dtype=FP32)
    Wq, Wk, Wv, Wo = (np.asarray(a, dtype=FP32) for a in (Wq, Wk, Wv, Wo))
    Wq_ind = np.asarray(Wq_ind, dtype=FP32)
    Wk_ind = np.asarray(Wk_ind, dtype=FP32)
    head_weights = np.asarray(head_weights, dtype=FP32)

    B, S, H = hidden_states.shape
    assert B == 1 and H == HIDDEN
    CS = H // N_CORES
    hidT = np.ascontiguousarray(hidden_states[0].T)
    # host-fused indexer weights (fp64 for exactness)
    Wfq = (Wq.astype(np.float64) @ Wq_ind.astype(np.float64)).astype(FP32)
    Wfk = (Wk.astype(np.float64) @ Wk_ind.astype(np.float64)).astype(FP32)

    # ---- launch A ----
    nca = _get("a", build_a, S, H, CS)
    ina = [
        {
            "hidT": hidT,
            "wq": np.ascontiguousarray(Wq[:, c * CS:(c + 1) * CS]),
            "wk": np.ascontiguousarray(Wk[:, c * CS:(c + 1) * CS]),
            "wv": np.ascontiguousarray(Wv[:, c * CS:(c + 1) * CS]),
            "wfq": np.ascontiguousarray(Wfq[:, c * CS:(c + 1) * CS]),
            "wfk": np.ascontiguousarray(Wfk[:, c * CS:(c + 1) * CS]),
        }
        for c in range(N_CORES)
    ]
    ra = _run(nca, ina)

    rel = np.zeros(S, dtype=np.float64)
    for c in range(N_CORES):
        rel += float(head_weights[c]) * ra[c]["rel"].astype(np.float64)
    # exp(-temp) scaling is monotone; irrelevant for top-k selection.

    k_sel = min(MAX_SELECTED, S)
    top_idx = np.argpartition(-rel, k_sel - 1)[:k_sel]
    selected = np.zeros(S, dtype=bool)
    selected[top_idx] = True

    # ---- launch B ----
    BIG = float(2 * S + 1024)
    hi = np.where(selected, BIG, np.arange(S, dtype=np.float64) + LOCAL_WINDOW)
    inb = [
        {
            "qTh": ra[c]["qT"],
            "kTh": ra[c]["kT"],
            "vTh": ra[c]["vT"],
            "woh": Wo[c * CS:(c + 1) * CS].astype(np.float16),
            "hivec": hi.astype(np.float16),
            "selv": selected.astype(np.float16),
        }
        for c in range(N_CORES)
    ]
    ncb = _get("b", build_b, S, H, NUM_HEADS // N_CORES, HEAD_DIM, LOCAL_WINDOW)
    rb = _run(ncb, inb)
    out = np.zeros((S, H), dtype=np.float32)
    for c in range(N_CORES):
        out += rb[c]["part"].astype(np.float32)
    return out.reshape(B, S, H)
